# revision 1
# baseline (speedup 1.0000x reference)
"""Trainium2 Bass kernel for windowed attention with LoRA + decomposed rel-pos bias.

Full-input contract: kernel(**inputs) takes the unsharded numpy inputs and
returns the full (64, 14, 14, 768) float32 output.

Strategy (8 NeuronCores, data-parallel over the 64-window batch, 8 windows/core):
  Host prep (numpy):
    - Fold LoRA into qkv weights:  Wq += lb_q@la_q, Wv += lb_v@la_v  (exact math).
    - Fold attention scale (2^-3, exact) into Wq / b_q; rel-pos tables get 1/scale.
    - Pre-transpose all weights + x so every on-chip matmul operand has its
      contraction dim on SBUF partitions (no on-chip transposes at all).
    - Gather rel_pos tables with the (q-k) index map; cast everything to bf16.
  On chip (per core, all SBUF resident):
    - qk projection -> per-(window, head) "augmented" q/k tiles [128, 196]:
      rows hold q (or k) in one 64-row half plus 14 rel-pos feature rows and
      14 one-hot rows so that ONE matmul per key-chunk produces
      q@k^T*scale + rel_h + rel_w directly in PSUM (K-augmentation trick).
    - exp on ScalarE (softmax without max-subtraction: logits are O(1)).
    - attn@v with an appended ones-column on v so the softmax denominator
      falls out of the same matmul; normalize with a reciprocal + DRAM-bounce
      DMA partition-broadcast + one VectorE multiply.
    - head-major attention interleaved with the rel-feature stage so VectorE
      copy work overlaps TensorE matmuls; projection reads a persistent
      all-window out2 tile allocated in the space freed by the xT pool.
"""

import numpy as np
import ml_dtypes

B_TOTAL = 64
NCORES = 8
BPC = B_TOTAL // NCORES  # windows per core
H = W = 14
N = H * W  # 196 tokens per window
DIM = 768
NH = 12
HD = 64
DC = DIM // 128  # 6 contraction chunks
NKT0, NKT1 = 128, N - 128  # key-token chunks (128 + 68)
SCALE = HD ** -0.5  # 0.125, exact power of two

# row maps inside the 128-partition augmented q/k tiles
# even head parity: q/k rows 0:64, relh/kh-onehot 64:78, zeros 78:96,
#                   relw/kw-onehot 96:110; contraction range [0:110)
# odd  head parity: relw/kw-onehot 0:14, zeros 14:32, relh/kh-onehot 32:46,
#                   zeros 46:64, q/k rows 64:128; contraction range [0:128)
K_EVEN = 110
K_ODD = 128

_NC_CACHE = {}


def build_module(debug=False):
    from contextlib import ExitStack

    import concourse.tile as tile
    from concourse import bacc, mybir

    f32 = mybir.dt.float32
    bf16 = mybir.dt.bfloat16
    AF = mybir.ActivationFunctionType
    ALU = mybir.AluOpType

    nc = bacc.Bacc(
        "TRN2", target_bir_lowering=False, debug=False, num_devices=NCORES
    )

    T = BPC * N  # 1568 tokens per core

    xT = nc.dram_tensor("xT", [DIM, T], bf16, kind="ExternalInput").ap()
    wqk = nc.dram_tensor("wqk", [DIM, 2 * DIM], bf16, kind="ExternalInput").ap()
    wv = nc.dram_tensor("wv", [DIM, DIM], bf16, kind="ExternalInput").ap()
    pw = nc.dram_tensor("pw", [DIM, DIM], bf16, kind="ExternalInput").ap()
    bqk = nc.dram_tensor("bqk", [2 * DIM], f32, kind="ExternalInput").ap()
    bv = nc.dram_tensor("bv", [DIM], bf16, kind="ExternalInput").ap()
    bp = nc.dram_tensor("bp", [DIM], bf16, kind="ExternalInput").ap()
    relh = nc.dram_tensor("relh", [HD, N], bf16, kind="ExternalInput").ap()
    relw = nc.dram_tensor("relw", [HD, N], bf16, kind="ExternalInput").ap()
    oh_e = nc.dram_tensor("oh_e", [46, N], bf16, kind="ExternalInput").ap()
    oh_o = nc.dram_tensor("oh_o", [64, N], bf16, kind="ExternalInput").ap()
    zer = nc.dram_tensor("zer", [18, N], bf16, kind="ExternalInput").ap()
    out = nc.dram_tensor("out", [T, DIM], f32, kind="ExternalOutput").ap()
    if debug:
        d_qaug = nc.dram_tensor(
            "d_qaug", [128, BPC * NH, N], bf16, kind="ExternalOutput"
        ).ap()
        d_kaug = nc.dram_tensor(
            "d_kaug", [128, BPC * NH, N], bf16, kind="ExternalOutput"
        ).ap()
        d_vall = nc.dram_tensor(
            "d_vall", [128, BPC * 2 * NH * (HD + 1)], bf16, kind="ExternalOutput"
        ).ap()
        d_o2 = nc.dram_tensor(
            "d_o2", [128, BPC, DC, N], bf16, kind="ExternalOutput"
        ).ap()

    with tile.TileContext(nc) as tc, ExitStack() as ctx:
        singles = ctx.enter_context(tc.tile_pool(name="singles", bufs=1))
        ps = ctx.enter_context(tc.tile_pool(name="ps", bufs=5, space="PSUM"))
        psd = ctx.enter_context(tc.tile_pool(name="psd", bufs=2, space="PSUM"))
        pdp = ctx.enter_context(tc.tile_pool(name="pdp", bufs=1, space="PSUM"))
        attn_pool = ctx.enter_context(tc.tile_pool(name="attn", bufs=3))
        r_pool = ctx.enter_context(tc.tile_pool(name="rp", bufs=2))
        osb_pool = ctx.enter_context(tc.tile_pool(name="osb", bufs=2))
        rd_pool = ctx.enter_context(tc.tile_pool(name="rd", bufs=2, space="DRAM"))
        xt_pool_cm = tc.tile_pool(name="xt", bufs=1)
        xt_pool = xt_pool_cm.__enter__()

        # ---- resident SBUF tensors ----
        wqk_sb = singles.tile([128, DC, 2 * DIM], bf16)
        wqk_r = wqk.rearrange("(c p) o -> p c o", p=128)
        for c in range(DC):
            nc.sync.dma_start(out=wqk_sb[:, c, :], in_=wqk_r[:, c, :])
        wv_sb = singles.tile([128, DC, DIM], bf16)
        nc.sync.dma_start(out=wv_sb[:], in_=wv.rearrange("(c p) o -> p c o", p=128))
        pw_sb = singles.tile([128, DC, DIM], bf16)
        nc.sync.dma_start(out=pw_sb[:], in_=pw.rearrange("(c p) o -> p c o", p=128))
        bqk_sb = singles.tile([128, 2 * DC], f32)
        nc.sync.dma_start(out=bqk_sb[:], in_=bqk.rearrange("(c p) -> p c", p=128))
        bv_sb = singles.tile([128, DIM], bf16)
        nc.sync.dma_start(out=bv_sb[:], in_=bv.unsqueeze(0).broadcast_to([128, DIM]))
        bp_sb = singles.tile([128, DIM], bf16)
        nc.sync.dma_start(out=bp_sb[:], in_=bp.unsqueeze(0).broadcast_to([128, DIM]))
        relh_sb = singles.tile([128, N], bf16)
        nc.sync.dma_start(out=relh_sb[0:64, :], in_=relh)
        nc.sync.dma_start(out=relh_sb[64:128, :], in_=relh)
        relw_sb = singles.tile([128, N], bf16)
        nc.sync.dma_start(out=relw_sb[0:64, :], in_=relw)
        nc.sync.dma_start(out=relw_sb[64:128, :], in_=relw)

        NPAIR = BPC * NH  # 96
        qaug = singles.tile([128, NPAIR, N], bf16)
        kaug = singles.tile([128, NPAIR, N], bf16)
        # [t-chunk partitions, window, chunk, head, hd+ones]
        vall = singles.tile([128, BPC, 2, NH, HD + 1], bf16)
        nc.vector.memset(vall[:, :, :, :, HD : HD + 1], 1.0)
        if debug:
            nc.vector.memset(qaug[:], 0.0)
            nc.vector.memset(kaug[:], 0.0)
            nc.vector.memset(vall[:], 0.0)
            nc.vector.memset(vall[:, :, :, :, HD : HD + 1], 1.0)

        # xT lives only through the projection phases; its pool is released
        # afterwards so the persistent out2 tile can reuse the space.
        xT_sb = xt_pool.tile([128, DC, T], bf16)
        xT_r = xT.rearrange("(c p) t -> p c t", p=128)
        for c in range(DC):
            nc.sync.dma_start(out=xT_sb[:, c, :], in_=xT_r[:, c, :])

        # views
        qv = qaug.rearrange(
            "p (b hh par) q -> p b hh par q", b=BPC, hh=NH // 2, par=2
        )
        qv6 = qaug.rearrange(
            "p (b hh par) (qh qw) -> p b hh par qh qw",
            b=BPC, hh=NH // 2, par=2, qh=H,
        )
        NPR = BPC * NH // 2  # 48 even/odd pair slots
        qpv = qaug.rearrange("p (pr par) q -> p pr par q", par=2)
        kpv = kaug.rearrange("p (pr par) q -> p pr par q", par=2)

        def bcast(src_ap, rows):
            return src_ap[0:rows].rearrange("j q -> j () q").broadcast_to(
                [rows, NPR, N]
            )

        nc.sync.dma_start(out=kpv[64:110, :, 0, :], in_=bcast(oh_e, 46))
        nc.sync.dma_start(out=kpv[0:64, :, 1, :], in_=bcast(oh_o, 64))
        nc.sync.dma_start(out=qpv[78:96, :, 0, :], in_=bcast(zer, 18))
        nc.sync.dma_start(out=qpv[14:32, :, 1, :], in_=bcast(zer, 18))
        nc.sync.dma_start(out=qpv[46:64, :, 1, :], in_=bcast(zer, 18))

        qp = qaug
        kp = kaug

        # ---- phase 1: q/k projection (two windows per psum tile) ----
        dest_v = [
            qaug.rearrange("p (b2 w2 h) q -> p b2 w2 h q", w2=2, h=NH),
            kaug.rearrange("p (b2 w2 h) q -> p b2 w2 h q", w2=2, h=NH),
        ]
        for b2 in range(BPC // 2):
            for oc in range(2 * DC):  # 6 q chunks then 6 k chunks
                p_qk = ps.tile([128, 512], f32, tag="ps")
                for dc in range(DC):
                    nc.tensor.matmul(
                        p_qk[:, 0 : 2 * N],
                        lhsT=wqk_sb[:, dc, oc * 128 : (oc + 1) * 128],
                        rhs=xT_sb[:, dc, 2 * b2 * N : (2 * b2 + 2) * N],
                        start=(dc == 0),
                        stop=(dc == DC - 1),
                    )
                is_q = oc < DC
                hh = (oc % DC) * 2
                dv = dest_v[0] if is_q else dest_v[1]
                for par in range(2):
                    h = hh + par
                    rows = slice(0, 64) if par == 0 else slice(64, 128)
                    nc.scalar.activation(
                        out=dv[rows, b2, :, h, :],
                        in_=p_qk[rows, 0 : 2 * N].rearrange("p (w q) -> p w q", w=2),
                        func=AF.Identity,
                        bias=bqk_sb[rows, oc : oc + 1],
                        scale=1.0,
                    )

        # ---- phase 1b: v projection (natural layout, per window) ----
        for b in range(BPC):
            for i in range(2):  # token chunk within window: 128 / 68
                tc_rows = NKT0 if i == 0 else NKT1
                t0 = b * N + i * 128
                for half in range(2):
                    p_v = ps.tile([128, 512], f32, tag="ps")
                    for dc in range(DC):
                        nc.tensor.matmul(
                            p_v[0:tc_rows, 0:384],
                            lhsT=xT_sb[:, dc, t0 : t0 + tc_rows],
                            rhs=wv_sb[:, dc, half * 384 : (half + 1) * 384],
                            start=(dc == 0),
                            stop=(dc == DC - 1),
                        )
                    nc.vector.tensor_tensor(
                        out=vall[0:tc_rows, b, i, 6 * half : 6 * half + 6, 0:HD],
                        in0=p_v[0:tc_rows, 0:384].rearrange("p (h d) -> p h d", h=6),
                        in1=bv_sb[0:tc_rows, half * 384 : (half + 1) * 384].rearrange(
                            "p (h d) -> p h d", h=6
                        ),
                        op=ALU.add,
                    )

        # xT no longer needed; free its zone for o2_all
        xt_pool_cm.__exit__(None, None, None)
        o2_pool = ctx.enter_context(tc.tile_pool(name="o2", bufs=1))
        o2_all = o2_pool.tile([128, DC, T], bf16)

        # ---- phases 2+3, head-PAIR major: rel features then attention
        #      for both parities of a chunk, sharing one AV psum tile ----
        def emit_rel(hx):
            par = hx % 2
            q_rows = slice(0, 64) if par == 0 else slice(64, 128)
            lh_base = 0 if par == 0 else 64
            relh_rows = slice(64, 78) if par == 0 else slice(32, 46)
            relw_rows = slice(96, 110) if par == 0 else slice(0, 14)
            relh_tp = (lh_base, 64 if par == 0 else 32)
            relw_tp = (lh_base, 96 if par == 0 else 0)
            hh, hp = hx // 2, hx % 2
            for g2 in range(H // 2):
                g0 = 2 * g2
                p_r = psd.tile([128, 4, 128], f32, tag="psd")
                for s in range(2):
                    g = g0 + s
                    nc.tensor.matmul(
                        p_r[relh_rows, s, 0 : BPC * W],
                        lhsT=relh_sb[q_rows, g * W : (g + 1) * W],
                        rhs=qv[q_rows, :, hh, hp, g * W : (g + 1) * W],
                        start=True,
                        stop=True,
                        tile_position=relh_tp,
                    )
                    nc.tensor.matmul(
                        p_r[relw_rows, 2 + s, 0 : BPC * W],
                        lhsT=relw_sb[q_rows, g * W : (g + 1) * W],
                        rhs=qv[q_rows, :, hh, hp, g : g + 13 * W + 1 : W],
                        start=True,
                        stop=True,
                        tile_position=relw_tp,
                    )
                nc.vector.tensor_copy(
                    out=qv6[relh_rows, :, hh, hp, g0 : g0 + 2, :],
                    in_=p_r[relh_rows, 0:2, 0 : BPC * W].rearrange(
                        "p s (b w) -> p b s w", b=BPC
                    ),
                )
                nc.vector.tensor_copy(
                    out=qv6[relw_rows, :, hh, hp, :, g0 : g0 + 2],
                    in_=p_r[relw_rows, 2:4, 0 : BPC * W].rearrange(
                        "p s (b q) -> p b q s", b=BPC
                    ),
                )

        def emit_qk_exp(b, hx, a_sb):
            par = hx % 2
            pair = b * NH + hx
            krange = slice(0, K_EVEN) if par == 0 else slice(0, K_ODD)
            p_a = ps.tile([128, 2, 256], f32, tag="ps")
            nc.tensor.matmul(
                p_a[:, 0, 0:N],
                lhsT=kp[krange, pair, 0:NKT0],
                rhs=qp[krange, pair, :],
                start=True,
                stop=True,
            )
            nc.tensor.matmul(
                p_a[0:NKT1, 1, 0:N],
                lhsT=kp[krange, pair, NKT0:N],
                rhs=qp[krange, pair, :],
                start=True,
                stop=True,
            )
            nc.scalar.activation(
                out=a_sb[:, 0, :], in_=p_a[:, 0, 0:N], func=AF.Exp, scale=1.0
            )
            nc.scalar.activation(
                out=a_sb[0:NKT1, 1, :],
                in_=p_a[0:NKT1, 1, 0:N],
                func=AF.Exp,
                scale=1.0,
            )

        def emit_denom(b, hx, a_sb, pdf, r_hh):
            par = hx % 2
            d_row = 64 if par == 0 else 0
            d_tp = (0, 64) if par == 0 else (0, 0)
            nc.tensor.matmul(
                pdf[d_row : d_row + 1, 0:N],
                lhsT=vall[0:NKT0, b, 0, hx, HD : HD + 1],
                rhs=a_sb[:, 0, :],
                start=True,
                stop=False,
                tile_position=d_tp,
            )
            nc.tensor.matmul(
                pdf[d_row : d_row + 1, 0:N],
                lhsT=vall[0:NKT1, b, 1, hx, HD : HD + 1],
                rhs=a_sb[0:NKT1, 1, :],
                start=False,
                stop=True,
                tile_position=d_tp,
            )
            with nc.allow_low_precision(reason="bf16 softmax recip"):
                nc.vector.reciprocal(
                    out=r_hh[d_row : d_row + 1, b, :],
                    in_=pdf[d_row : d_row + 1, 0:N],
                )

        def emit_av(b, hx, a_sb, p_o):
            par = hx % 2
            rows = slice(0, 64) if par == 0 else slice(64, 128)
            av_tp = (0, 0) if par == 0 else (0, 64)
            nc.tensor.matmul(
                p_o[rows, 0:N],
                lhsT=vall[0:NKT0, b, 0, hx, 0:HD],
                rhs=a_sb[:, 0, :],
                start=True,
                stop=False,
                tile_position=av_tp,
                skip_group_check=True,
            )
            nc.tensor.matmul(
                p_o[rows, 0:N],
                lhsT=vall[0:NKT1, b, 1, hx, 0:HD],
                rhs=a_sb[0:NKT1, 1, :],
                start=False,
                stop=True,
                tile_position=av_tp,
                skip_group_check=True,
            )

        for hh in range(NH // 2):
            h0, h1 = 2 * hh, 2 * hh + 1
            emit_rel(h0)
            emit_rel(h1)
            r_hh = r_pool.tile([65, BPC, N], bf16, tag="rw")
            for b in range(BPC):
                a_sb0 = attn_pool.tile([128, 2, N], bf16, tag="a0")
                a_sb1 = attn_pool.tile([128, 2, N], bf16, tag="a1")
                emit_qk_exp(b, h0, a_sb0)
                emit_qk_exp(b, h1, a_sb1)
                p_dd = pdp.tile([128, 4, 128], f32, tag="pdp")
                pdf = p_dd.rearrange("p s c -> p (s c)")
                emit_denom(b, h0, a_sb0, pdf, r_hh)
                emit_denom(b, h1, a_sb1, pdf, r_hh)
                p_o = ps.tile([128, 512], f32, tag="ps")
                emit_av(b, h0, a_sb0, p_o)
                emit_av(b, h1, a_sb1, p_o)
                nc.scalar.activation(
                    out=o2_all[:, hh, b * N : (b + 1) * N],
                    in_=p_o[:, 0:N],
                    func=AF.Copy,
                    scale=1.0,
                )

            # broadcast reciprocals for both parities (DRAM bounce)
            dd = rd_pool.tile([2, BPC, N], bf16, tag="rd")
            rb_hh = r_pool.tile([128, BPC, N], bf16, tag="rb")
            nc.sync.dma_start(out=dd[0:1, :, :], in_=r_hh[64:65, :, :])
            nc.sync.dma_start(out=dd[1:2, :, :], in_=r_hh[0:1, :, :])
            nc.sync.dma_start(
                out=rb_hh[0:64, :, :], in_=dd[0:1, :, :].broadcast_to([64, BPC, N])
            )
            nc.sync.dma_start(
                out=rb_hh[64:128, :, :],
                in_=dd[1:2, :, :].broadcast_to([64, BPC, N]),
            )
            nc.vector.tensor_tensor(
                out=o2_all[:, hh, :].rearrange("p (b q) -> p b q", b=BPC),
                in0=o2_all[:, hh, :].rearrange("p (b q) -> p b q", b=BPC),
                in1=rb_hh[:, :, :],
                op=ALU.mult,
            )

        # ---- phase 4: projection over global 128-token chunks ----
        NT_CH = (T + 127) // 128  # 13
        for j in range(NT_CH):
            t0 = j * 128
            tc_rows = min(128, T - t0)
            o_sb = osb_pool.tile([128, DIM], f32, tag="osb")
            for half in range(2):
                p_p = ps.tile([128, 512], f32, tag="ps")
                for cc in range(DC):
                    nc.tensor.matmul(
                        p_p[0:tc_rows, 0:384],
                        lhsT=o2_all[:, cc, t0 : t0 + tc_rows],
                        rhs=pw_sb[:, cc, half * 384 : (half + 1) * 384],
                        start=(cc == 0),
                        stop=(cc == DC - 1),
                    )
                nc.vector.tensor_tensor(
                    out=o_sb[0:tc_rows, half * 384 : (half + 1) * 384],
                    in0=p_p[0:tc_rows, 0:384],
                    in1=bp_sb[0:tc_rows, half * 384 : (half + 1) * 384],
                    op=ALU.add,
                )
            nc.sync.dma_start(
                out=out[t0 : t0 + tc_rows, :],
                in_=o_sb[0:tc_rows, :],
            )

        if debug:
            nc.sync.dma_start(out=d_qaug, in_=qaug[:])
            nc.sync.dma_start(out=d_kaug, in_=kaug[:])
            nc.sync.dma_start(
                out=d_vall, in_=vall.rearrange("p a b c d -> p (a b c d)")
            )

    nc.finalize()
    return nc


def _host_prep(inputs):
    bf16 = ml_dtypes.bfloat16
    x = np.asarray(inputs["x"], np.float32)
    qkv_w = np.asarray(inputs["qkv_w"], np.float32)
    qkv_b = np.asarray(inputs["qkv_b"], np.float32)
    proj_w = np.asarray(inputs["proj_w"], np.float32)
    proj_b = np.asarray(inputs["proj_b"], np.float32)
    la_q = np.asarray(inputs["la_q"], np.float32)
    lb_q = np.asarray(inputs["lb_q"], np.float32)
    la_v = np.asarray(inputs["la_v"], np.float32)
    lb_v = np.asarray(inputs["lb_v"], np.float32)
    rel_pos_h = np.asarray(inputs["rel_pos_h"], np.float32)
    rel_pos_w = np.asarray(inputs["rel_pos_w"], np.float32)

    Wq = qkv_w[:DIM] + lb_q @ la_q
    Wk = qkv_w[DIM : 2 * DIM]
    Wv = qkv_w[2 * DIM :] + lb_v @ la_v

    wqk_host = np.ascontiguousarray(
        np.concatenate([SCALE * Wq, Wk], 0).T.astype(bf16)
    )
    wv_host = np.ascontiguousarray(Wv.T.astype(bf16))
    pw_host = np.ascontiguousarray(proj_w.T.astype(bf16))
    bqk_host = np.concatenate([SCALE * qkv_b[:DIM], qkv_b[DIM : 2 * DIM]]).astype(
        np.float32
    )
    bv_host = np.ascontiguousarray(qkv_b[2 * DIM :].astype(bf16))
    bp_host = np.ascontiguousarray(proj_b.astype(bf16))

    idx = np.arange(H)[:, None] - np.arange(H)[None, :] + (H - 1)
    Rh = rel_pos_h[idx]  # [qh, kh_j, hd]
    Rw = rel_pos_w[idx]  # [qw, kw_j, hd]
    relh_host = np.ascontiguousarray(
        (Rh / SCALE).transpose(2, 0, 1).reshape(HD, N).astype(bf16)
    )
    relw_host = np.ascontiguousarray(
        (Rw / SCALE).transpose(2, 0, 1).reshape(HD, N).astype(bf16)
    )

    kt = np.arange(N)
    oh_kh = (kt[None, :] // W == np.arange(H)[:, None]).astype(bf16)  # [14, 196]
    oh_kw = (kt[None, :] % W == np.arange(W)[:, None]).astype(bf16)
    z18 = np.zeros((18, N), bf16)
    oh_e_host = np.ascontiguousarray(np.concatenate([oh_kh, z18, oh_kw], 0))
    oh_o_host = np.ascontiguousarray(
        np.concatenate([oh_kw, z18, oh_kh, z18], 0)
    )

    shared = {
        "wqk": wqk_host,
        "wv": wv_host,
        "pw": pw_host,
        "bqk": bqk_host,
        "bv": bv_host,
        "bp": bp_host,
        "relh": relh_host,
        "relw": relw_host,
        "oh_e": oh_e_host,
        "oh_o": oh_o_host,
        "zer": z18,
    }

    x_flat = x.reshape(B_TOTAL, N, DIM)
    in_maps = []
    for c in range(NCORES):
        xc = x_flat[c * BPC : (c + 1) * BPC].reshape(BPC * N, DIM)
        xT_c = np.ascontiguousarray(xc.T.astype(bf16))
        m = dict(shared)
        m["xT"] = xT_c
        in_maps.append(m)
    return in_maps


def kernel(**inputs):
    from concourse import bass_utils

    if "nc" not in _NC_CACHE:
        _NC_CACHE["nc"] = build_module()
    nc = _NC_CACHE["nc"]
    in_maps = _host_prep(inputs)
    res = bass_utils.run_bass_kernel_spmd(
        nc, in_maps, core_ids=list(range(NCORES))
    )
    outs = [r["out"].reshape(BPC, H, W, DIM) for r in res.results]
    return np.concatenate(outs, 0)



# revision 50
# speedup vs baseline: 1.5389x; 1.5389x over previous
"""Trainium2 Bass kernel for windowed attention with LoRA + decomposed rel-pos bias.

Full-input contract: kernel(**inputs) takes the unsharded numpy inputs and
returns the full (64, 14, 14, 768) float32 output.

Strategy (8 NeuronCores, data-parallel over the 64-window batch, 8 windows/core):
  Host prep (numpy):
    - Fold LoRA into qkv weights:  Wq += lb_q@la_q, Wv += lb_v@la_v  (exact math).
    - Fold attention scale (2^-3, exact) into Wq / b_q; rel-pos tables get 1/scale.
    - Pre-transpose all weights + x so every on-chip matmul operand has its
      contraction dim on SBUF partitions (no on-chip transposes at all).
    - Gather rel_pos tables with the (q-k) index map; cast everything to bf16.
  On chip (per core, all SBUF resident):
    - one-hot / zero K-augmentation rows are built on-chip (memset + DVE
      broadcast copies from small staged tables) instead of huge broadcast
      DMAs; all DMA transfers share one global engine pool, so this matters.
    - qk projection -> per-(window, head) "augmented" q/k tiles [128, 196]:
      rows hold q (or k) in one 64-row half plus 14 rel-pos feature rows and
      14 one-hot rows so that ONE matmul per key-chunk produces
      q@k^T*scale + rel_h + rel_w directly in PSUM (K-augmentation trick).
    - key-token chunks are 0:128 and 68:196 (overlapping) so both exp
      activations cover full 128 partitions with valid data; the second
      attn@v matmul contracts only rows 60:128 (tokens 128:196).
    - exp on ScalarE (softmax without max-subtraction: logits are O(1)).
    - attn@v: even-parity head appends the ones-column to its lhsT so the
      softmax denominator falls out of the same matmul (65-row output);
      odd-parity head emits a separate 1-col ones matmul (its output rows
      sit at base 64 where a 65-wide lhsT is not placeable). Both denoms
      land in one PSUM tile -> ONE reciprocal per (window, head-pair).
    - qk projection and attention are fused per head-pair so TensorE never
      drains: chunk oc=hh (q) and oc=6+hh (k) are projected, then rel
      features + attention for heads 2hh,2hh+1 run while the next pair's
      projection matmuls queue behind them.
    - normalization via DRAM-bounce partition-broadcast of reciprocals,
      one bounce per head-pair; one VectorE multiply.
    - engine balance: ScalarE = projection copies + exp; VectorE = bias
      adds, reciprocals, normalize mult, part of rel copies; GpSimd (Pool)
      = attention-output copies + rest of rel copies.
"""

import numpy as np
import ml_dtypes

B_TOTAL = 64
NCORES = 8
BPC = B_TOTAL // NCORES  # windows per core
H = W = 14
N = H * W  # 196 tokens per window
DIM = 768
NH = 12
HD = 64
DC = DIM // 128  # 6 contraction chunks
SCALE = HD ** -0.5  # 0.125, exact power of two

# key-token chunks: chunk0 = tokens 0:128, chunk1 = tokens 68:196 (overlap);
# attn@v contracts chunk1 rows 60:128 only (tokens 128:196).
CH1 = 68  # chunk1 token offset
AV0 = 68  # rows contracted from chunk0 (tokens 0:68); chunk1 covers 68:196

# row maps inside the 128-partition augmented q/k tiles
# even head parity: q/k rows 0:64, relh/kh-onehot 64:78, zeros 78:96,
#                   relw/kw-onehot 96:110; contraction range [0:110)
# odd  head parity: relw/kw-onehot 0:14, zeros 14:32, relh/kh-onehot 32:46,
#                   zeros 46:64, q/k rows 64:128; contraction range [0:128)
K_EVEN = 110
K_ODD = 128

VW = 66  # vall row width: [ones, v(64), ones]

# fp8 (e4m3) DoubleRow mode for the q/k projections: 0 = off (bf16),
# 1 = k only, 2 = q and k. Weights/x are pre-scaled by 64 on the host
# (e4m3 min-normal is 2^-6; w ~ 0.02); q/k land in SBUF scaled by 64 and
# the 1/4096 comes out exactly in the exp() scale. Measured end-to-end
# rel-err: off 3.9e-3, k8 1.13e-2, qk8 1.67e-2 (gate 2e-2).
FP8_QK = 2

_NC_CACHE = {}


def build_module(debug=False):
    from contextlib import ExitStack

    import concourse.tile as tile
    from concourse import bacc, mybir

    f32 = mybir.dt.float32
    bf16 = mybir.dt.bfloat16
    f8 = mybir.dt.float8e4
    PM = mybir.MatmulPerfMode
    AF = mybir.ActivationFunctionType
    ALU = mybir.AluOpType

    nc = bacc.Bacc(
        "TRN2", target_bir_lowering=False, debug=False, num_devices=NCORES
    )

    T = BPC * N  # 1568 tokens per core

    xT = nc.dram_tensor("xT", [DIM, T], bf16, kind="ExternalInput").ap()
    n8 = DIM * FP8_QK  # fp8 output-channel count (k only, or q and k)
    if FP8_QK:
        xT8 = nc.dram_tensor("xT8", [DIM, T], f8, kind="ExternalInput").ap()
        wqk8 = nc.dram_tensor("wqk8", [DIM, n8], f8, kind="ExternalInput").ap()
    if FP8_QK < 2:
        wqk = nc.dram_tensor(
            "wqk", [DIM, 2 * DIM - n8], bf16, kind="ExternalInput"
        ).ap()
    wv = nc.dram_tensor("wv", [DIM, DIM], bf16, kind="ExternalInput").ap()
    pw = nc.dram_tensor("pw", [DIM, DIM], bf16, kind="ExternalInput").ap()
    bqk = nc.dram_tensor("bqk", [2 * DIM], f32, kind="ExternalInput").ap()
    bv = nc.dram_tensor("bv", [DIM], bf16, kind="ExternalInput").ap()
    bp = nc.dram_tensor("bp", [DIM], bf16, kind="ExternalInput").ap()
    relh = nc.dram_tensor("relh", [HD, N], bf16, kind="ExternalInput").ap()
    relw = nc.dram_tensor("relw", [HD, N], bf16, kind="ExternalInput").ap()
    oh_e = nc.dram_tensor("oh_e", [46, N], bf16, kind="ExternalInput").ap()
    oh_o = nc.dram_tensor("oh_o", [64, N], bf16, kind="ExternalInput").ap()
    out = nc.dram_tensor("out", [T, DIM], f32, kind="ExternalOutput").ap()

    with tile.TileContext(nc) as tc, ExitStack() as ctx:
        singles = ctx.enter_context(tc.tile_pool(name="singles", bufs=1))
        po_pool = ctx.enter_context(tc.tile_pool(name="po", bufs=2, space="PSUM"))
        pa_cm = tc.tile_pool(name="pa", bufs=6, space="PSUM")
        pa_pool = pa_cm.__enter__()
        attn_pool = ctx.enter_context(tc.tile_pool(name="attn", bufs=8))
        osb_pool = ctx.enter_context(tc.tile_pool(name="osb", bufs=2))
        xt_pool_cm = tc.tile_pool(name="xt", bufs=1)
        xt_pool = xt_pool_cm.__enter__()

        # ---- resident SBUF tensors; DMA order = phase order ----
        # qk projection (fp8) runs first, v projection second, so its inputs
        # (wqk8/xT8/rel tables) load first and v's bf16 x streams in behind.
        bqk_sb = singles.tile([128, 2 * DC], f32)
        nc.sync.dma_start(out=bqk_sb[:], in_=bqk.rearrange("(c p) -> p c", p=128))
        ohe_sb = singles.tile([46, N], bf16)
        nc.sync.dma_start(out=ohe_sb[:], in_=oh_e)
        oho_sb = singles.tile([64, N], bf16)
        nc.sync.dma_start(out=oho_sb[:], in_=oh_o)
        if FP8_QK:
            wqk8_sb = singles.tile([128, DC, n8], f8)
            wqk8_r = wqk8.rearrange("(c p) o -> p c o", p=128)
            xT8_sb = xt_pool.tile([128, DC, T], f8)
            xT8_r = xT8.rearrange("(c p) t -> p c t", p=128)
            for c in range(DC):
                nc.sync.dma_start(out=wqk8_sb[:, c, :], in_=wqk8_r[:, c, :])
                nc.sync.dma_start(
                    out=xT8_sb[:, c, 0 : 2 * N], in_=xT8_r[:, c, 0 : 2 * N]
                )
            for c in range(DC):
                nc.sync.dma_start(
                    out=xT8_sb[:, c, 2 * N :], in_=xT8_r[:, c, 2 * N :]
                )
        if FP8_QK < 2:
            wqk_sb = singles.tile([128, DC, 2 * DIM - n8], bf16)
            wqk_r = wqk.rearrange("(c p) o -> p c o", p=128)
            for c in range(DC):
                nc.sync.dma_start(out=wqk_sb[:, c, :], in_=wqk_r[:, c, :])
        relh_sb = singles.tile([128, N], bf16)
        nc.sync.dma_start(out=relh_sb[0:64, :], in_=relh)
        nc.sync.dma_start(out=relh_sb[64:128, :], in_=relh)
        relw_sb = singles.tile([128, N], bf16)
        nc.sync.dma_start(out=relw_sb[0:64, :], in_=relw)
        nc.sync.dma_start(out=relw_sb[64:128, :], in_=relw)
        bv_sb = singles.tile([128, DIM], bf16)
        nc.sync.dma_start(out=bv_sb[:], in_=bv.unsqueeze(0).broadcast_to([128, DIM]))
        wv_sb = singles.tile([128, DC, DIM], bf16)
        wv_r = wv.rearrange("(c p) o -> p c o", p=128)
        xT_sb = xt_pool.tile([128, DC, T], bf16)
        xT_r = xT.rearrange("(c p) t -> p c t", p=128)
        for c in range(DC):
            nc.sync.dma_start(out=wv_sb[:, c, :], in_=wv_r[:, c, :])
            nc.sync.dma_start(out=xT_sb[:, c, :], in_=xT_r[:, c, :])
        pw_sb = singles.tile([128, DC, DIM], bf16)
        nc.sync.dma_start(out=pw_sb[:], in_=pw.rearrange("(c p) o -> p c o", p=128))
        bp_sb = singles.tile([128, DIM], bf16)
        nc.sync.dma_start(out=bp_sb[:], in_=bp.unsqueeze(0).broadcast_to([128, DIM]))

        NPAIR = BPC * NH  # 96
        qaug = singles.tile([128, NPAIR, N], bf16)
        kaug = singles.tile([128, NPAIR, N], bf16)
        # [t-chunk partitions, window, chunk, head, ones+hd+ones]
        vall = singles.tile([128, BPC, 2, NH, VW], bf16)

        # ---- on-chip prep ----
        # one-hot K rows built by broadcast copies; q-side garbage bands are
        # zeroed by DMA broadcasts from oh_e's zero block (GPSIMD cannot
        # touch PSUM, and a full-tile memset would serialize ahead of the
        # projection copies). Bands are only read by attention (qk matmuls),
        # so these DMAs just need to land before then.
        nc.gpsimd.memset(vall[:, :, :, :, 0:1], 1.0)
        nc.gpsimd.memset(vall[:, :, :, :, VW - 1 : VW], 1.0)
        ones64 = singles.tile([128, HD], bf16)
        nc.gpsimd.memset(ones64[:], 1.0)

        NPR = BPC * NH // 2  # 48 even/odd pair slots
        qv = qaug.rearrange(
            "p (b hh par) q -> p b hh par q", b=BPC, hh=NH // 2, par=2
        )
        qv6 = qaug.rearrange(
            "p (b hh par) (qh qw) -> p b hh par qh qw",
            b=BPC, hh=NH // 2, par=2, qh=H,
        )
        kpv = kaug.rearrange("p (pr par) q -> p pr par q", par=2)
        qpv = qaug.rearrange("p (pr par) q -> p pr par q", par=2)
        nc.gpsimd.tensor_copy(
            out=kpv[64:110, :, 0, :],
            in_=ohe_sb.rearrange("j q -> j () q").broadcast_to([46, NPR, N]),
        )
        nc.gpsimd.tensor_copy(
            out=kpv[0:64, :, 1, :],
            in_=oho_sb.rearrange("j q -> j () q").broadcast_to([64, NPR, N]),
        )
        zsrc = oh_e[14:32, :].rearrange("j q -> j () q")
        nc.sync.dma_start(out=qpv[78:96, :, 0, :], in_=zsrc.broadcast_to([18, NPR, N]))
        nc.sync.dma_start(out=qpv[14:32, :, 1, :], in_=zsrc.broadcast_to([18, NPR, N]))
        nc.sync.dma_start(out=qpv[46:64, :, 1, :], in_=zsrc.broadcast_to([18, NPR, N]))

        qp = qaug
        kp = kaug

        # ---- phase 1b: v projection groups (interleaved into phase 1) ----
        def emit_v_group(b, i, half):
            t0 = b * N + (0 if i == 0 else CH1)
            p_v = pa_pool.tile([128, 512], f32, tag="pa", name="p_v")
            for dc in range(DC):
                nc.tensor.matmul(
                    p_v[:, 0:384],
                    lhsT=xT_sb[:, dc, t0 : t0 + 128],
                    rhs=wv_sb[:, dc, half * 384 : (half + 1) * 384],
                    start=(dc == 0),
                    stop=(dc == DC - 1),
                )
            nc.vector.tensor_tensor(
                out=vall[:, b, i, 6 * half : 6 * half + 6, 1 : 1 + HD],
                in0=p_v[:, 0:384].rearrange("p (h d) -> p h d", h=6),
                in1=bv_sb[:, half * 384 : (half + 1) * 384].rearrange(
                    "p (h d) -> p h d", h=6
                ),
                op=ALU.add,
            )

        v_groups = [
            (b, i, half) for b in range(BPC) for i in range(2) for half in range(2)
        ]
        v_pos = [0]

        def emit_v(n):
            for _ in range(n):
                if v_pos[0] < len(v_groups):
                    emit_v_group(*v_groups[v_pos[0]])
                    v_pos[0] += 1



        # views for phase-1 destinations
        dest_v = [
            qaug.rearrange("p (b2 w2 h) q -> p b2 w2 h q", w2=2, h=NH),
            kaug.rearrange("p (b2 w2 h) q -> p b2 w2 h q", w2=2, h=NH),
        ]

        def emit_qkproj_group(oc, b2):
            is_q = oc < DC
            hh = (oc % DC) * 2
            dv = dest_v[0] if is_q else dest_v[1]
            p_qk = pa_pool.tile([128, 512], f32, tag="pa")
            use8 = FP8_QK == 2 or (FP8_QK == 1 and not is_q)
            if use8:
                oc8 = oc if FP8_QK == 2 else oc - DC
                for dr in range(DC // 2):
                    nc.tensor.matmul(
                        p_qk[:, 0 : 2 * N],
                        lhsT=wqk8_sb[
                            :, 2 * dr : 2 * dr + 2, oc8 * 128 : (oc8 + 1) * 128
                        ],
                        rhs=xT8_sb[:, 2 * dr : 2 * dr + 2, 2 * b2 * N : (2 * b2 + 2) * N],
                        start=(dr == 0),
                        stop=(dr == DC // 2 - 1),
                        perf_mode=PM.DoubleRow,
                    )
            else:
                oc_b = oc if FP8_QK == 0 else oc  # bf16 table holds q chunks only
                for dc in range(DC):
                    nc.tensor.matmul(
                        p_qk[:, 0 : 2 * N],
                        lhsT=wqk_sb[:, dc, oc_b * 128 : (oc_b + 1) * 128],
                        rhs=xT_sb[:, dc, 2 * b2 * N : (2 * b2 + 2) * N],
                        start=(dc == 0),
                        stop=(dc == DC - 1),
                    )
            nc.scalar.activation(
                out=dv[0:64, b2, :, hh, :],
                in_=p_qk[0:64, 0 : 2 * N].rearrange("p (w q) -> p w q", w=2),
                func=AF.Identity,
                bias=bqk_sb[0:64, oc : oc + 1],
                scale=1.0,
            )
            nc.vector.tensor_tensor(
                out=dv[64:128, b2, :, hh + 1, :],
                in0=p_qk[64:128, 0 : 2 * N].rearrange("p (w q) -> p w q", w=2),
                in1=bqk_sb[64:128, oc : oc + 1]
                .rearrange("p x -> p x ()")
                .broadcast_to([64, 2, N]),
                op=ALU.add,
            )

        def do_copy(ei, out, in_):
            if ei == 0:
                nc.vector.tensor_copy(out=out, in_=in_)
            elif ei == 1:
                nc.gpsimd.tensor_copy(out=out, in_=in_)
            else:
                nc.scalar.activation(out=out, in_=in_, func=AF.Copy, scale=1.0)

        # ---- rel features (phase 2), emitted as pumpable 4g blocks ----
        rel_eng = [2, 2, 0]  # Act-leaning DVE/Act mix; GPSIMD cannot touch PSUM
        rel_ctr = [0]

        def make_rel_group(hx, blk, kind):
            par = hx % 2
            q_rows = slice(0, 64) if par == 0 else slice(64, 128)
            lh_base = 0 if par == 0 else 64
            relh_rows = slice(64, 78) if par == 0 else slice(32, 46)
            relw_rows = slice(96, 110) if par == 0 else slice(0, 14)
            relh_tp = (lh_base, 64 if par == 0 else 32)
            relw_tp = (lh_base, 96 if par == 0 else 0)
            hh, hp = hx // 2, hx % 2
            g0 = 4 * blk
            ng = min(4, H - g0)

            def emit():
                p_r = pa_pool.tile([128, 4, 128], f32, tag="pa", name="p_r")
                rows = relh_rows if kind == 0 else relw_rows
                for s in range(ng):
                    g = g0 + s
                    if kind == 0:
                        nc.tensor.matmul(
                            p_r[rows, s, 0 : BPC * W],
                            lhsT=relh_sb[q_rows, g * W : (g + 1) * W],
                            rhs=qv[q_rows, :, hh, hp, g * W : (g + 1) * W],
                            start=True,
                            stop=True,
                            tile_position=relh_tp,
                        )
                    else:
                        nc.tensor.matmul(
                            p_r[rows, s, 0 : BPC * W],
                            lhsT=relw_sb[q_rows, g * W : (g + 1) * W],
                            rhs=qv[q_rows, :, hh, hp, g : g + 13 * W + 1 : W],
                            start=True,
                            stop=True,
                            tile_position=relw_tp,
                        )
                ei = rel_eng[rel_ctr[0] % len(rel_eng)]
                rel_ctr[0] += 1
                if kind == 0:
                    do_copy(
                        ei,
                        qv6[rows, :, hh, hp, g0 : g0 + ng, :],
                        p_r[rows, 0:ng, 0 : BPC * W].rearrange(
                            "p s (b w) -> p b s w", b=BPC
                        ),
                    )
                else:
                    do_copy(
                        ei,
                        qv6[rows, :, hh, hp, :, g0 : g0 + ng],
                        p_r[rows, 0:ng, 0 : BPC * W].rearrange(
                            "p s (b q) -> p b q s", b=BPC
                        ),
                    )

            return emit

        pending_rel = []

        def pump(n):
            for _ in range(min(n, len(pending_rel))):
                pending_rel.pop(0)()

        # ---- attention inner stages ----
        def emit_qk_exp(b, hh, a_sb):
            p_a = pb_pool.tile([128, 4, 256], f32, tag="pab")
            for par in range(2):
                hx = 2 * hh + par
                pair = b * NH + hx
                krange = slice(0, K_EVEN) if par == 0 else slice(0, K_ODD)
                nc.tensor.matmul(
                    p_a[:, 2 * par, 0:N],
                    lhsT=kp[krange, pair, 0:128],
                    rhs=qp[krange, pair, :],
                    start=True,
                    stop=True,
                )
                nc.tensor.matmul(
                    p_a[:, 2 * par + 1, 0:N],
                    lhsT=kp[krange, pair, CH1:N],
                    rhs=qp[krange, pair, :],
                    start=True,
                    stop=True,
                )
            nc.scalar.activation(
                out=a_sb[:],
                in_=p_a[:, :, 0:N],
                func=AF.Exp,
                scale=1.0 / (64.0 ** FP8_QK),
            )

        def emit_av(b, hh, a_sb, p_o):
            h0, h1 = 2 * hh, 2 * hh + 1
            # slot 0: attn@v for both parities (rows 0:64 even, 64:128 odd);
            # slot 1: softmax denominators broadcast to the same rows via a
            # 64-wide ones block, so ONE divide normalizes and writes o2.
            for par, h in ((0, h0), (1, h1)):
                tp = (0, 0) if par == 0 else (0, 64)
                rows = slice(0, 64) if par == 0 else slice(64, 128)
                nc.tensor.matmul(
                    p_o[rows, 0, 0:N],
                    lhsT=vall[0:AV0, b, 0, h, 1 : 1 + HD],
                    rhs=a_sb[0:AV0, 2 * par, :],
                    start=True,
                    stop=False,
                    tile_position=tp,
                    skip_group_check=True,
                )
                nc.tensor.matmul(
                    p_o[rows, 0, 0:N],
                    lhsT=vall[:, b, 1, h, 1 : 1 + HD],
                    rhs=a_sb[:, 2 * par + 1, :],
                    start=False,
                    stop=True,
                    tile_position=tp,
                    skip_group_check=True,
                )
                nc.tensor.matmul(
                    p_o[rows, 1, 0:N],
                    lhsT=ones64[0:AV0, :],
                    rhs=a_sb[0:AV0, 2 * par, :],
                    start=True,
                    stop=False,
                    tile_position=tp,
                    skip_group_check=True,
                )
                nc.tensor.matmul(
                    p_o[rows, 1, 0:N],
                    lhsT=ones64[:],
                    rhs=a_sb[:, 2 * par + 1, :],
                    start=False,
                    stop=True,
                    tile_position=tp,
                    skip_group_check=True,
                )

        # ---- phase 1: qk + v projections interleaved, rel pumped in ----
        gctr = 0
        for hh in range(NH // 2):
            for oc in (hh, DC + hh):
                for b2 in range(BPC // 2):
                    emit_qkproj_group(oc, b2)
                    pump(2)
                    gctr += 1
                    if gctr > 16:
                        emit_v((gctr - 16) - v_pos[0])
                if oc < DC and b2 == BPC // 2 - 1:
                    for hx in (2 * hh, 2 * hh + 1):
                        for blk in range(4):
                            for kind in range(2):
                                pending_rel.append(make_rel_group(hx, blk, kind))

        # drain remaining rel groups and v
        pump(len(pending_rel))
        emit_v(len(v_groups))
        pa_cm.__exit__(None, None, None)
        pb_cm = tc.tile_pool(name="pb", bufs=3, space="PSUM")
        pb_pool = pb_cm.__enter__()

        # xT no longer needed; free its zone for the persistent o2 tile
        xt_pool_cm.__exit__(None, None, None)
        o2_pool = ctx.enter_context(tc.tile_pool(name="o2", bufs=1))
        o2_all = o2_pool.tile([128, DC, T], bf16)

        # ---- phase 3: attention, per head pair ----
        for hh in range(NH // 2):
            inflight = []

            def drain_one():
                pb, xa = inflight.pop(0)
                p_o = po_pool.tile([128, 2, 256], f32, tag="po")
                emit_av(pb, hh, xa, p_o)
                # hw: no divide ALU op, and TensorTensor reads at most one
                # input from PSUM -> reciprocal to SBUF, then multiply
                den_sb = attn_pool.tile([128, N], f32, tag="den")
                nc.vector.reciprocal(out=den_sb[:], in_=p_o[:, 1, 0:N])
                with nc.allow_low_precision(reason="bf16 softmax"):
                    nc.vector.tensor_tensor(
                        out=o2_all[:, hh, pb * N : (pb + 1) * N],
                        in0=p_o[:, 0, 0:N],
                        in1=den_sb[:],
                        op=ALU.mult,
                    )

            for b in range(BPC):
                a_sb = attn_pool.tile([128, 4, N], bf16, tag="a0")
                emit_qk_exp(b, hh, a_sb)
                pump(2)
                inflight.append((b, a_sb))
                if len(inflight) > 2:
                    drain_one()
            while inflight:
                drain_one()

        # ---- phase 4: projection over global 128-token chunks ----
        pb_cm.__exit__(None, None, None)
        ps4_cm = tc.tile_pool(name="ps4", bufs=2, space="PSUM")
        ps4 = ps4_cm.__enter__()
        NT_CH = (T + 127) // 128  # 13
        for j in range(NT_CH):
            t0 = j * 128
            tc_rows = min(128, T - t0)
            o_sb = osb_pool.tile([128, DIM], f32, tag="osb")
            for half in range(2):
                p_p = ps4.tile([128, 512], f32, tag="ps")
                for cc in range(DC):
                    nc.tensor.matmul(
                        p_p[0:tc_rows, 0:384],
                        lhsT=o2_all[:, cc, t0 : t0 + tc_rows],
                        rhs=pw_sb[:, cc, half * 384 : (half + 1) * 384],
                        start=(cc == 0),
                        stop=(cc == DC - 1),
                    )
                nc.vector.tensor_tensor(
                    out=o_sb[0:tc_rows, half * 384 : (half + 1) * 384],
                    in0=p_p[0:tc_rows, 0:384],
                    in1=bp_sb[0:tc_rows, half * 384 : (half + 1) * 384],
                    op=ALU.add,
                )
            nc.sync.dma_start(
                out=out[t0 : t0 + tc_rows, :],
                in_=o_sb[0:tc_rows, :],
            )
        ps4_cm.__exit__(None, None, None)

    nc.finalize()
    return nc


def _host_prep(inputs):
    bf16 = ml_dtypes.bfloat16
    f8 = ml_dtypes.float8_e4m3fn
    x = np.asarray(inputs["x"], np.float32)
    qkv_w = np.asarray(inputs["qkv_w"], np.float32)
    qkv_b = np.asarray(inputs["qkv_b"], np.float32)
    proj_w = np.asarray(inputs["proj_w"], np.float32)
    proj_b = np.asarray(inputs["proj_b"], np.float32)
    la_q = np.asarray(inputs["la_q"], np.float32)
    lb_q = np.asarray(inputs["lb_q"], np.float32)
    la_v = np.asarray(inputs["la_v"], np.float32)
    lb_v = np.asarray(inputs["lb_v"], np.float32)
    rel_pos_h = np.asarray(inputs["rel_pos_h"], np.float32)
    rel_pos_w = np.asarray(inputs["rel_pos_w"], np.float32)

    Wq = qkv_w[:DIM] + lb_q @ la_q
    Wk = qkv_w[DIM : 2 * DIM]
    Wv = qkv_w[2 * DIM :] + lb_v @ la_v

    s8 = 64.0
    if FP8_QK == 2:
        wqk8_host = np.ascontiguousarray(
            np.concatenate([s8 * SCALE * Wq, s8 * Wk], 0).T.astype(f8)
        )
        bqk_host = (
            s8 * np.concatenate([SCALE * qkv_b[:DIM], qkv_b[DIM : 2 * DIM]])
        ).astype(np.float32)
    elif FP8_QK == 1:
        wqk8_host = np.ascontiguousarray((s8 * Wk).T.astype(f8))
        wqk_host = np.ascontiguousarray((SCALE * Wq).T.astype(bf16))
        bqk_host = np.concatenate(
            [SCALE * qkv_b[:DIM], s8 * qkv_b[DIM : 2 * DIM]]
        ).astype(np.float32)
    else:
        wqk_host = np.ascontiguousarray(
            np.concatenate([SCALE * Wq, Wk], 0).T.astype(bf16)
        )
        bqk_host = np.concatenate(
            [SCALE * qkv_b[:DIM], qkv_b[DIM : 2 * DIM]]
        ).astype(np.float32)
    wv_host = np.ascontiguousarray(Wv.T.astype(bf16))
    pw_host = np.ascontiguousarray(proj_w.T.astype(bf16))
    bv_host = np.ascontiguousarray(qkv_b[2 * DIM :].astype(bf16))
    bp_host = np.ascontiguousarray(proj_b.astype(bf16))

    idx = np.arange(H)[:, None] - np.arange(H)[None, :] + (H - 1)
    Rh = rel_pos_h[idx]  # [qh, kh_j, hd]
    Rw = rel_pos_w[idx]  # [qw, kw_j, hd]
    rscale = s8 if FP8_QK else 1.0
    relh_host = np.ascontiguousarray(
        (rscale * Rh / SCALE).transpose(2, 0, 1).reshape(HD, N).astype(bf16)
    )
    relw_host = np.ascontiguousarray(
        (rscale * Rw / SCALE).transpose(2, 0, 1).reshape(HD, N).astype(bf16)
    )

    kt = np.arange(N)
    oh_kh = (kt[None, :] // W == np.arange(H)[:, None]).astype(bf16)  # [14, 196]
    oh_kw = (kt[None, :] % W == np.arange(W)[:, None]).astype(bf16)
    z18 = np.zeros((18, N), bf16)
    oh_e_host = np.ascontiguousarray(np.concatenate([oh_kh, z18, oh_kw], 0))
    oh_o_host = np.ascontiguousarray(
        np.concatenate([oh_kw, z18, oh_kh, z18], 0)
    )

    shared = {
        "wv": wv_host,
        "pw": pw_host,
        "bqk": bqk_host,
        "bv": bv_host,
        "bp": bp_host,
        "relh": relh_host,
        "relw": relw_host,
        "oh_e": oh_e_host,
        "oh_o": oh_o_host,
    }
    if FP8_QK:
        shared["wqk8"] = wqk8_host
    if FP8_QK < 2:
        shared["wqk"] = wqk_host

    x_flat = x.reshape(B_TOTAL, N, DIM)
    in_maps = []
    for c in range(NCORES):
        xc = x_flat[c * BPC : (c + 1) * BPC].reshape(BPC * N, DIM)
        xT_c = np.ascontiguousarray(xc.T.astype(bf16))
        m = dict(shared)
        m["xT"] = xT_c
        if FP8_QK:
            m["xT8"] = np.ascontiguousarray(xT_c.astype(f8))
        in_maps.append(m)
    return in_maps


def kernel(**inputs):
    from concourse import bass_utils

    if "nc" not in _NC_CACHE:
        _NC_CACHE["nc"] = build_module()
    nc = _NC_CACHE["nc"]
    in_maps = _host_prep(inputs)
    res = bass_utils.run_bass_kernel_spmd(
        nc, in_maps, core_ids=list(range(NCORES))
    )
    outs = [r["out"].reshape(BPC, H, W, DIM) for r in res.results]
    return np.concatenate(outs, 0)


# revision 60
# speedup vs baseline: 1.5781x; 1.0255x over previous
"""Trainium2 Bass kernel for windowed attention with LoRA + decomposed rel-pos bias.

Full-input contract: kernel(**inputs) takes the unsharded numpy inputs and
returns the full (64, 14, 14, 768) float32 output.

Strategy (8 NeuronCores, data-parallel over the 64-window batch, 8 windows/core):
  Host prep (numpy):
    - Fold LoRA into qkv weights (exact); fold the 1/8 attention scale into
      Wq/bq; pre-transpose weights and x so every on-chip matmul contracts
      over SBUF partitions; gather rel-pos tables by the (q-k) index map.
    - q/k projection runs in fp8-e4m3 DoubleRow mode (two 128-row k-tiles
      per pass, 4x bf16 MACs): weights and x are scaled by 64 on the host
      (e4m3 min-normal vs w~0.02); the exact 1/4096 comes back out in the
      exp() activation scale. v and the output projection stay bf16 -- their
      quantization error would land on the output linearly, while q/k noise
      averages out across the softmax (measured end-to-end rel-err 1.7e-2
      vs the 2e-2 gate; bf16-only is 3.9e-3, set FP8_QK = 0 to fall back).
  On chip (per core, single fused pipeline, all SBUF resident):
    - one-hot / zero K-augmentation rows are built on-chip (GpSimd broadcast
      copies) or by three small DMA broadcasts (partition bases 78/14/46 are
      not engine-addressable); no big DMA broadcasts, no full-tile memset.
    - qk projection -> per-(window, head) "augmented" q/k tiles [128, 196]:
      rows hold q (or k) plus 14 rel-pos feature rows and 14 one-hot rows so
      ONE matmul per key-chunk produces q@k^T*scale + rel_h + rel_w directly
      in PSUM (K-augmentation trick). v projection groups are interleaved
      into the qk loop so its PSUM->SBUF copies hide under v's PE time.
    - rel features are computed in pumpable 4-row blocks interleaved between
      projection groups; copies rotate over ScalarE/VectorE (GPSIMD cannot
      access PSUM on real hardware).
    - key-token chunks 0:128 and 68:196 overlap so both exp activations
      cover 128 fully-valid partitions; attn@v contracts chunk0 rows 0:68
      and chunk1 rows 0:128 (disjoint tokens, partition bases 0).
    - one exp per (window, head-pair) over a 4-slot PSUM tile; softmax skips
      max-subtraction (logits are O(1)).
    - attn@v writes both parities into PSUM slot 0 (rows 0:64 / 64:128) and
      a 64-wide ones-block matmul broadcasts both softmax denominators onto
      the same rows in slot 1: one VectorE reciprocal + one multiply
      normalizes and writes o2 -- no cross-partition broadcast, no DRAM
      bounce (the hardware has no divide ALU op and TensorTensor may read
      only one PSUM operand, hence recip+mult through SBUF).
    - attention runs software-pipelined 2 deep (qk/exp of iteration b+2
      issue before attn@v of iteration b); all 1-bank PSUM tiles share one
      6-buffer pool so rotation never stalls TensorE.
    - output projection over 13 aligned 128-token chunks, bias added on
      VectorE, PSUM pool swapped per phase (6x1-bank -> 3x2-bank -> 2x1).
"""

import numpy as np
import ml_dtypes

B_TOTAL = 64
NCORES = 8
BPC = B_TOTAL // NCORES  # windows per core
H = W = 14
N = H * W  # 196 tokens per window
DIM = 768
NH = 12
HD = 64
DC = DIM // 128  # 6 contraction chunks
SCALE = HD ** -0.5  # 0.125, exact power of two

# key-token chunks: chunk0 = tokens 0:128, chunk1 = tokens 68:196 (overlap);
# attn@v contracts chunk1 rows 60:128 only (tokens 128:196).
CH1 = 68  # chunk1 token offset
AV0 = 68  # rows contracted from chunk0 (tokens 0:68); chunk1 covers 68:196

# row maps inside the 128-partition augmented q/k tiles
# even head parity: q/k rows 0:64, relh/kh-onehot 64:78, zeros 78:96,
#                   relw/kw-onehot 96:110; contraction range [0:110)
# odd  head parity: relw/kw-onehot 0:14, zeros 14:32, relh/kh-onehot 32:46,
#                   zeros 46:64, q/k rows 64:128; contraction range [0:128)
K_EVEN = 110
K_ODD = 128

VW = 66  # vall row width: [ones, v(64), ones]

# fp8 (e4m3) DoubleRow mode for the q/k projections: 0 = off (bf16),
# 1 = k only, 2 = q and k. Weights/x are pre-scaled by 64 on the host
# (e4m3 min-normal is 2^-6; w ~ 0.02); q/k land in SBUF scaled by 64 and
# the 1/4096 comes out exactly in the exp() scale. Measured end-to-end
# rel-err: off 3.9e-3, k8 1.13e-2, qk8 1.67e-2 (gate 2e-2).
FP8_QK = 2

_NC_CACHE = {}


def build_module(debug=False):
    from contextlib import ExitStack

    import concourse.tile as tile
    from concourse import bacc, mybir

    f32 = mybir.dt.float32
    bf16 = mybir.dt.bfloat16
    f8 = mybir.dt.float8e4
    PM = mybir.MatmulPerfMode
    AF = mybir.ActivationFunctionType
    ALU = mybir.AluOpType

    nc = bacc.Bacc(
        "TRN2", target_bir_lowering=False, debug=False, num_devices=NCORES
    )

    T = BPC * N  # 1568 tokens per core

    xT = nc.dram_tensor("xT", [DIM, T], bf16, kind="ExternalInput").ap()
    n8 = DIM * FP8_QK  # fp8 output-channel count (k only, or q and k)
    if FP8_QK:
        xT8 = nc.dram_tensor("xT8", [DIM, T], f8, kind="ExternalInput").ap()
        wqk8 = nc.dram_tensor("wqk8", [DIM, n8], f8, kind="ExternalInput").ap()
    if FP8_QK < 2:
        wqk = nc.dram_tensor(
            "wqk", [DIM, 2 * DIM - n8], bf16, kind="ExternalInput"
        ).ap()
    wv = nc.dram_tensor("wv", [DIM, DIM], bf16, kind="ExternalInput").ap()
    pw = nc.dram_tensor("pw", [DIM, DIM], bf16, kind="ExternalInput").ap()
    bqk = nc.dram_tensor("bqk", [2 * DIM], f32, kind="ExternalInput").ap()
    bv = nc.dram_tensor("bv", [DIM], bf16, kind="ExternalInput").ap()
    bp = nc.dram_tensor("bp", [DIM], bf16, kind="ExternalInput").ap()
    relh = nc.dram_tensor("relh", [HD, N], bf16, kind="ExternalInput").ap()
    relw = nc.dram_tensor("relw", [HD, N], bf16, kind="ExternalInput").ap()
    oh_e = nc.dram_tensor("oh_e", [46, N], bf16, kind="ExternalInput").ap()
    oh_o = nc.dram_tensor("oh_o", [64, N], bf16, kind="ExternalInput").ap()
    out = nc.dram_tensor("out", [T, DIM], f32, kind="ExternalOutput").ap()

    with tile.TileContext(nc) as tc, ExitStack() as ctx:
        singles = ctx.enter_context(tc.tile_pool(name="singles", bufs=1))
        po_pool = ctx.enter_context(tc.tile_pool(name="po", bufs=2, space="PSUM"))
        pa_cm = tc.tile_pool(name="pa", bufs=6, space="PSUM")
        pa_pool = pa_cm.__enter__()
        attn_pool = ctx.enter_context(tc.tile_pool(name="attn", bufs=8))
        osb_pool = ctx.enter_context(tc.tile_pool(name="osb", bufs=2))
        xt_pool_cm = tc.tile_pool(name="xt", bufs=1)
        xt_pool = xt_pool_cm.__enter__()

        # ---- resident SBUF tensors; DMA order = phase order ----
        # qk projection (fp8) runs first, v projection second, so its inputs
        # (wqk8/xT8/rel tables) load first and v's bf16 x streams in behind.
        bqk_sb = singles.tile([128, 2 * DC], f32)
        nc.sync.dma_start(out=bqk_sb[:], in_=bqk.rearrange("(c p) -> p c", p=128))
        ohe_sb = singles.tile([46, N], bf16)
        nc.sync.dma_start(out=ohe_sb[:], in_=oh_e)
        oho_sb = singles.tile([64, N], bf16)
        nc.sync.dma_start(out=oho_sb[:], in_=oh_o)
        if FP8_QK:
            wqk8_sb = singles.tile([128, DC, n8], f8)
            wqk8_r = wqk8.rearrange("(c p) o -> p c o", p=128)
            xT8_sb = xt_pool.tile([128, DC, T], f8)
            xT8_r = xT8.rearrange("(c p) t -> p c t", p=128)
            for c in range(DC):
                nc.sync.dma_start(out=wqk8_sb[:, c, :], in_=wqk8_r[:, c, :])
                nc.scalar.dma_start(
                    out=xT8_sb[:, c, 0 : 2 * N], in_=xT8_r[:, c, 0 : 2 * N]
                )
            for c in range(DC):
                nc.sync.dma_start(
                    out=xT8_sb[:, c, 2 * N :], in_=xT8_r[:, c, 2 * N :]
                )
        if FP8_QK < 2:
            wqk_sb = singles.tile([128, DC, 2 * DIM - n8], bf16)
            wqk_r = wqk.rearrange("(c p) o -> p c o", p=128)
            for c in range(DC):
                nc.sync.dma_start(out=wqk_sb[:, c, :], in_=wqk_r[:, c, :])
        relh_sb = singles.tile([128, N], bf16)
        nc.sync.dma_start(out=relh_sb[0:64, :], in_=relh)
        nc.sync.dma_start(out=relh_sb[64:128, :], in_=relh)
        relw_sb = singles.tile([128, N], bf16)
        nc.sync.dma_start(out=relw_sb[0:64, :], in_=relw)
        nc.sync.dma_start(out=relw_sb[64:128, :], in_=relw)
        bv_sb = singles.tile([128, DIM], bf16)
        nc.sync.dma_start(out=bv_sb[:], in_=bv.unsqueeze(0).broadcast_to([128, DIM]))
        wv_sb = singles.tile([128, DC, DIM], bf16)
        wv_r = wv.rearrange("(c p) o -> p c o", p=128)
        xT_sb = xt_pool.tile([128, DC, T], bf16)
        xT_r = xT.rearrange("(c p) t -> p c t", p=128)
        for c in range(DC):
            nc.sync.dma_start(out=wv_sb[:, c, :], in_=wv_r[:, c, :])
            nc.sync.dma_start(out=xT_sb[:, c, :], in_=xT_r[:, c, :])
        pw_sb = singles.tile([128, DC, DIM], bf16)
        nc.sync.dma_start(out=pw_sb[:], in_=pw.rearrange("(c p) o -> p c o", p=128))
        bp_sb = singles.tile([128, DIM], bf16)
        nc.sync.dma_start(out=bp_sb[:], in_=bp.unsqueeze(0).broadcast_to([128, DIM]))

        NPAIR = BPC * NH  # 96
        qaug = singles.tile([128, NPAIR, N], bf16)
        kaug = singles.tile([128, NPAIR, N], bf16)
        # [t-chunk partitions, window, chunk, head, ones+hd+ones]
        vall = singles.tile([128, BPC, 2, NH, VW], bf16)

        # ---- on-chip prep ----
        # one-hot K rows built by broadcast copies; q-side garbage bands are
        # zeroed by DMA broadcasts from oh_e's zero block (GPSIMD cannot
        # touch PSUM, and a full-tile memset would serialize ahead of the
        # projection copies). Bands are only read by attention (qk matmuls),
        # so these DMAs just need to land before then.
        ones64 = singles.tile([128, HD], bf16)
        nc.gpsimd.memset(ones64[:], 1.0)

        NPR = BPC * NH // 2  # 48 even/odd pair slots
        qv = qaug.rearrange(
            "p (b hh par) q -> p b hh par q", b=BPC, hh=NH // 2, par=2
        )
        qv6 = qaug.rearrange(
            "p (b hh par) (qh qw) -> p b hh par qh qw",
            b=BPC, hh=NH // 2, par=2, qh=H,
        )
        kpv = kaug.rearrange("p (pr par) q -> p pr par q", par=2)
        qpv = qaug.rearrange("p (pr par) q -> p pr par q", par=2)
        nc.gpsimd.tensor_copy(
            out=kpv[64:110, :, 0, :],
            in_=ohe_sb.rearrange("j q -> j () q").broadcast_to([46, NPR, N]),
        )
        nc.gpsimd.tensor_copy(
            out=kpv[0:64, :, 1, :],
            in_=oho_sb.rearrange("j q -> j () q").broadcast_to([64, NPR, N]),
        )
        zsrc = oh_e[14:32, :].rearrange("j q -> j () q")
        nc.sync.dma_start(out=qpv[78:96, :, 0, :], in_=zsrc.broadcast_to([18, NPR, N]))
        nc.sync.dma_start(out=qpv[14:32, :, 1, :], in_=zsrc.broadcast_to([18, NPR, N]))
        nc.sync.dma_start(out=qpv[46:64, :, 1, :], in_=zsrc.broadcast_to([18, NPR, N]))

        qp = qaug
        kp = kaug

        # ---- phase 1b: v projection groups (interleaved into phase 1) ----
        def emit_v_group(b, i, half):
            t0 = b * N + (0 if i == 0 else CH1)
            p_v = pa_pool.tile([128, 512], f32, tag="pa", name="p_v")
            for dc in range(DC):
                nc.tensor.matmul(
                    p_v[:, 0:384],
                    lhsT=xT_sb[:, dc, t0 : t0 + 128],
                    rhs=wv_sb[:, dc, half * 384 : (half + 1) * 384],
                    start=(dc == 0),
                    stop=(dc == DC - 1),
                )
            nc.vector.tensor_tensor(
                out=vall[:, b, i, 6 * half : 6 * half + 6, 1 : 1 + HD],
                in0=p_v[:, 0:384].rearrange("p (h d) -> p h d", h=6),
                in1=bv_sb[:, half * 384 : (half + 1) * 384].rearrange(
                    "p (h d) -> p h d", h=6
                ),
                op=ALU.add,
            )

        v_groups = [
            (b, i, half) for b in range(BPC) for i in range(2) for half in range(2)
        ]
        v_pos = [0]

        def emit_v(n):
            for _ in range(n):
                if v_pos[0] < len(v_groups):
                    emit_v_group(*v_groups[v_pos[0]])
                    v_pos[0] += 1



        # views for phase-1 destinations
        dest_v = [
            qaug.rearrange("p (b2 w2 h) q -> p b2 w2 h q", w2=2, h=NH),
            kaug.rearrange("p (b2 w2 h) q -> p b2 w2 h q", w2=2, h=NH),
        ]

        def emit_qkproj_group(oc, b2):
            is_q = oc < DC
            hh = (oc % DC) * 2
            dv = dest_v[0] if is_q else dest_v[1]
            p_qk = pa_pool.tile([128, 512], f32, tag="pa")
            use8 = FP8_QK == 2 or (FP8_QK == 1 and not is_q)
            if use8:
                oc8 = oc if FP8_QK == 2 else oc - DC
                for dr in range(DC // 2):
                    nc.tensor.matmul(
                        p_qk[:, 0 : 2 * N],
                        lhsT=wqk8_sb[
                            :, 2 * dr : 2 * dr + 2, oc8 * 128 : (oc8 + 1) * 128
                        ],
                        rhs=xT8_sb[:, 2 * dr : 2 * dr + 2, 2 * b2 * N : (2 * b2 + 2) * N],
                        start=(dr == 0),
                        stop=(dr == DC // 2 - 1),
                        perf_mode=PM.DoubleRow,
                    )
            else:
                oc_b = oc if FP8_QK == 0 else oc  # bf16 table holds q chunks only
                for dc in range(DC):
                    nc.tensor.matmul(
                        p_qk[:, 0 : 2 * N],
                        lhsT=wqk_sb[:, dc, oc_b * 128 : (oc_b + 1) * 128],
                        rhs=xT_sb[:, dc, 2 * b2 * N : (2 * b2 + 2) * N],
                        start=(dc == 0),
                        stop=(dc == DC - 1),
                    )
            nc.scalar.activation(
                out=dv[0:64, b2, :, hh, :],
                in_=p_qk[0:64, 0 : 2 * N].rearrange("p (w q) -> p w q", w=2),
                func=AF.Identity,
                bias=bqk_sb[0:64, oc : oc + 1],
                scale=1.0,
            )
            nc.vector.tensor_tensor(
                out=dv[64:128, b2, :, hh + 1, :],
                in0=p_qk[64:128, 0 : 2 * N].rearrange("p (w q) -> p w q", w=2),
                in1=bqk_sb[64:128, oc : oc + 1]
                .rearrange("p x -> p x ()")
                .broadcast_to([64, 2, N]),
                op=ALU.add,
            )

        def do_copy(ei, out, in_):
            if ei == 0:
                nc.vector.tensor_copy(out=out, in_=in_)
            elif ei == 1:
                nc.gpsimd.tensor_copy(out=out, in_=in_)
            else:
                nc.scalar.activation(out=out, in_=in_, func=AF.Copy, scale=1.0)

        # ---- rel features (phase 2), emitted as pumpable 4g blocks ----
        rel_eng = [2, 2, 0, 0]  # DVE/Act mix; GPSIMD cannot touch PSUM
        rel_ctr = [0]

        def make_rel_group(hx, blk, kind):
            par = hx % 2
            q_rows = slice(0, 64) if par == 0 else slice(64, 128)
            lh_base = 0 if par == 0 else 64
            relh_rows = slice(64, 78) if par == 0 else slice(32, 46)
            relw_rows = slice(96, 110) if par == 0 else slice(0, 14)
            relh_tp = (lh_base, 64 if par == 0 else 32)
            relw_tp = (lh_base, 96 if par == 0 else 0)
            hh, hp = hx // 2, hx % 2
            g0 = 4 * blk
            ng = min(4, H - g0)

            def emit():
                p_r = pa_pool.tile([128, 4, 128], f32, tag="pa", name="p_r")
                rows = relh_rows if kind == 0 else relw_rows
                for s in range(ng):
                    g = g0 + s
                    if kind == 0:
                        nc.tensor.matmul(
                            p_r[rows, s, 0 : BPC * W],
                            lhsT=relh_sb[q_rows, g * W : (g + 1) * W],
                            rhs=qv[q_rows, :, hh, hp, g * W : (g + 1) * W],
                            start=True,
                            stop=True,
                            tile_position=relh_tp,
                        )
                    else:
                        nc.tensor.matmul(
                            p_r[rows, s, 0 : BPC * W],
                            lhsT=relw_sb[q_rows, g * W : (g + 1) * W],
                            rhs=qv[q_rows, :, hh, hp, g : g + 13 * W + 1 : W],
                            start=True,
                            stop=True,
                            tile_position=relw_tp,
                        )
                ei = rel_eng[rel_ctr[0] % len(rel_eng)]
                rel_ctr[0] += 1
                if kind == 0:
                    do_copy(
                        ei,
                        qv6[rows, :, hh, hp, g0 : g0 + ng, :],
                        p_r[rows, 0:ng, 0 : BPC * W].rearrange(
                            "p s (b w) -> p b s w", b=BPC
                        ),
                    )
                else:
                    do_copy(
                        ei,
                        qv6[rows, :, hh, hp, :, g0 : g0 + ng],
                        p_r[rows, 0:ng, 0 : BPC * W].rearrange(
                            "p s (b q) -> p b q s", b=BPC
                        ),
                    )

            return emit

        pending_rel = []

        def pump(n):
            for _ in range(min(n, len(pending_rel))):
                pending_rel.pop(0)()

        # ---- attention inner stages ----
        def emit_qk_exp(b, hh, a_sb):
            p_a = pb_pool.tile([128, 4, 256], f32, tag="pab")
            for par in range(2):
                hx = 2 * hh + par
                pair = b * NH + hx
                krange = slice(0, K_EVEN) if par == 0 else slice(0, K_ODD)
                nc.tensor.matmul(
                    p_a[:, 2 * par, 0:N],
                    lhsT=kp[krange, pair, 0:128],
                    rhs=qp[krange, pair, :],
                    start=True,
                    stop=True,
                )
                nc.tensor.matmul(
                    p_a[:, 2 * par + 1, 0:N],
                    lhsT=kp[krange, pair, CH1:N],
                    rhs=qp[krange, pair, :],
                    start=True,
                    stop=True,
                )
            nc.scalar.activation(
                out=a_sb[:],
                in_=p_a[:, :, 0:N],
                func=AF.Exp,
                scale=1.0 / (64.0 ** FP8_QK),
            )

        def emit_av(b, hh, a_sb, p_o):
            h0, h1 = 2 * hh, 2 * hh + 1
            # slot 0: attn@v for both parities (rows 0:64 even, 64:128 odd);
            # slot 1: softmax denominators broadcast to the same rows via a
            # 64-wide ones block, so ONE divide normalizes and writes o2.
            for par, h in ((0, h0), (1, h1)):
                tp = (0, 0) if par == 0 else (0, 64)
                rows = slice(0, 64) if par == 0 else slice(64, 128)
                nc.tensor.matmul(
                    p_o[rows, 0, 0:N],
                    lhsT=vall[0:AV0, b, 0, h, 1 : 1 + HD],
                    rhs=a_sb[0:AV0, 2 * par, :],
                    start=True,
                    stop=False,
                    tile_position=tp,
                    skip_group_check=True,
                )
                nc.tensor.matmul(
                    p_o[rows, 0, 0:N],
                    lhsT=vall[:, b, 1, h, 1 : 1 + HD],
                    rhs=a_sb[:, 2 * par + 1, :],
                    start=False,
                    stop=True,
                    tile_position=tp,
                    skip_group_check=True,
                )
                nc.tensor.matmul(
                    p_o[rows, 1, 0:N],
                    lhsT=ones64[0:AV0, :],
                    rhs=a_sb[0:AV0, 2 * par, :],
                    start=True,
                    stop=False,
                    tile_position=tp,
                    skip_group_check=True,
                )
                nc.tensor.matmul(
                    p_o[rows, 1, 0:N],
                    lhsT=ones64[:],
                    rhs=a_sb[:, 2 * par + 1, :],
                    start=False,
                    stop=True,
                    tile_position=tp,
                    skip_group_check=True,
                )

        # ---- phase 1: qk + v projections interleaved, rel pumped in ----
        gctr = 0
        for hh in range(NH // 2):
            for oc in (hh, DC + hh):
                for b2 in range(BPC // 2):
                    emit_qkproj_group(oc, b2)
                    pump(2)
                    gctr += 1
                    if gctr > 16:
                        emit_v((gctr - 16) - v_pos[0])
                if oc < DC and b2 == BPC // 2 - 1:
                    for hx in (2 * hh, 2 * hh + 1):
                        for blk in range(4):
                            for kind in range(2):
                                pending_rel.append(make_rel_group(hx, blk, kind))

        # drain remaining rel groups and v
        pump(len(pending_rel))
        emit_v(len(v_groups))
        pa_cm.__exit__(None, None, None)
        pb_cm = tc.tile_pool(name="pb", bufs=3, space="PSUM")
        pb_pool = pb_cm.__enter__()

        # xT no longer needed; free its zone for the persistent o2 tile
        xt_pool_cm.__exit__(None, None, None)
        o2_pool = ctx.enter_context(tc.tile_pool(name="o2", bufs=1))
        o2_all = o2_pool.tile([128, DC, T], bf16)

        # ---- phase 3: attention, per head pair ----
        for hh in range(NH // 2):
            inflight = []

            def drain_one():
                pb, xa = inflight.pop(0)
                p_o = po_pool.tile([128, 2, 256], f32, tag="po")
                emit_av(pb, hh, xa, p_o)
                # hw: no divide ALU op, and TensorTensor reads at most one
                # input from PSUM -> reciprocal to SBUF, then multiply
                den_sb = attn_pool.tile([128, N], f32, tag="den")
                nc.vector.reciprocal(out=den_sb[:], in_=p_o[:, 1, 0:N])
                # (recip stays on DVE: Act is exp-saturated in attention)
                with nc.allow_low_precision(reason="bf16 softmax"):
                    nc.vector.tensor_tensor(
                        out=o2_all[:, hh, pb * N : (pb + 1) * N],
                        in0=p_o[:, 0, 0:N],
                        in1=den_sb[:],
                        op=ALU.mult,
                    )

            for b in range(BPC):
                a_sb = attn_pool.tile([128, 4, N], bf16, tag="a0")
                emit_qk_exp(b, hh, a_sb)
                pump(2)
                inflight.append((b, a_sb))
                if len(inflight) > 2:
                    drain_one()
            while inflight:
                drain_one()

        # ---- phase 4: projection over global 128-token chunks ----
        pb_cm.__exit__(None, None, None)
        ps4_cm = tc.tile_pool(name="ps4", bufs=2, space="PSUM")
        ps4 = ps4_cm.__enter__()
        NT_CH = (T + 127) // 128  # 13
        for j in range(NT_CH):
            t0 = j * 128
            tc_rows = min(128, T - t0)
            o_sb = osb_pool.tile([128, DIM], f32, tag="osb")
            for half in range(2):
                p_p = ps4.tile([128, 512], f32, tag="ps")
                for cc in range(DC):
                    nc.tensor.matmul(
                        p_p[0:tc_rows, 0:384],
                        lhsT=o2_all[:, cc, t0 : t0 + tc_rows],
                        rhs=pw_sb[:, cc, half * 384 : (half + 1) * 384],
                        start=(cc == 0),
                        stop=(cc == DC - 1),
                    )
                nc.vector.tensor_tensor(
                    out=o_sb[0:tc_rows, half * 384 : (half + 1) * 384],
                    in0=p_p[0:tc_rows, 0:384],
                    in1=bp_sb[0:tc_rows, half * 384 : (half + 1) * 384],
                    op=ALU.add,
                )
            nc.sync.dma_start(
                out=out[t0 : t0 + tc_rows, :],
                in_=o_sb[0:tc_rows, :],
            )
        ps4_cm.__exit__(None, None, None)

    nc.finalize()
    return nc


def _host_prep(inputs):
    bf16 = ml_dtypes.bfloat16
    f8 = ml_dtypes.float8_e4m3fn
    x = np.asarray(inputs["x"], np.float32)
    qkv_w = np.asarray(inputs["qkv_w"], np.float32)
    qkv_b = np.asarray(inputs["qkv_b"], np.float32)
    proj_w = np.asarray(inputs["proj_w"], np.float32)
    proj_b = np.asarray(inputs["proj_b"], np.float32)
    la_q = np.asarray(inputs["la_q"], np.float32)
    lb_q = np.asarray(inputs["lb_q"], np.float32)
    la_v = np.asarray(inputs["la_v"], np.float32)
    lb_v = np.asarray(inputs["lb_v"], np.float32)
    rel_pos_h = np.asarray(inputs["rel_pos_h"], np.float32)
    rel_pos_w = np.asarray(inputs["rel_pos_w"], np.float32)

    Wq = qkv_w[:DIM] + lb_q @ la_q
    Wk = qkv_w[DIM : 2 * DIM]
    Wv = qkv_w[2 * DIM :] + lb_v @ la_v

    s8 = 64.0
    if FP8_QK == 2:
        wqk8_host = np.ascontiguousarray(
            np.concatenate([s8 * SCALE * Wq, s8 * Wk], 0).T.astype(f8)
        )
        bqk_host = (
            s8 * np.concatenate([SCALE * qkv_b[:DIM], qkv_b[DIM : 2 * DIM]])
        ).astype(np.float32)
    elif FP8_QK == 1:
        wqk8_host = np.ascontiguousarray((s8 * Wk).T.astype(f8))
        wqk_host = np.ascontiguousarray((SCALE * Wq).T.astype(bf16))
        bqk_host = np.concatenate(
            [SCALE * qkv_b[:DIM], s8 * qkv_b[DIM : 2 * DIM]]
        ).astype(np.float32)
    else:
        wqk_host = np.ascontiguousarray(
            np.concatenate([SCALE * Wq, Wk], 0).T.astype(bf16)
        )
        bqk_host = np.concatenate(
            [SCALE * qkv_b[:DIM], qkv_b[DIM : 2 * DIM]]
        ).astype(np.float32)
    wv_host = np.ascontiguousarray(Wv.T.astype(bf16))
    pw_host = np.ascontiguousarray(proj_w.T.astype(bf16))
    bv_host = np.ascontiguousarray(qkv_b[2 * DIM :].astype(bf16))
    bp_host = np.ascontiguousarray(proj_b.astype(bf16))

    idx = np.arange(H)[:, None] - np.arange(H)[None, :] + (H - 1)
    Rh = rel_pos_h[idx]  # [qh, kh_j, hd]
    Rw = rel_pos_w[idx]  # [qw, kw_j, hd]
    rscale = s8 if FP8_QK else 1.0
    relh_host = np.ascontiguousarray(
        (rscale * Rh / SCALE).transpose(2, 0, 1).reshape(HD, N).astype(bf16)
    )
    relw_host = np.ascontiguousarray(
        (rscale * Rw / SCALE).transpose(2, 0, 1).reshape(HD, N).astype(bf16)
    )

    kt = np.arange(N)
    oh_kh = (kt[None, :] // W == np.arange(H)[:, None]).astype(bf16)  # [14, 196]
    oh_kw = (kt[None, :] % W == np.arange(W)[:, None]).astype(bf16)
    z18 = np.zeros((18, N), bf16)
    oh_e_host = np.ascontiguousarray(np.concatenate([oh_kh, z18, oh_kw], 0))
    oh_o_host = np.ascontiguousarray(
        np.concatenate([oh_kw, z18, oh_kh, z18], 0)
    )

    shared = {
        "wv": wv_host,
        "pw": pw_host,
        "bqk": bqk_host,
        "bv": bv_host,
        "bp": bp_host,
        "relh": relh_host,
        "relw": relw_host,
        "oh_e": oh_e_host,
        "oh_o": oh_o_host,
    }
    if FP8_QK:
        shared["wqk8"] = wqk8_host
    if FP8_QK < 2:
        shared["wqk"] = wqk_host

    x_flat = x.reshape(B_TOTAL, N, DIM)
    in_maps = []
    for c in range(NCORES):
        xc = x_flat[c * BPC : (c + 1) * BPC].reshape(BPC * N, DIM)
        xT_c = np.ascontiguousarray(xc.T.astype(bf16))
        m = dict(shared)
        m["xT"] = xT_c
        if FP8_QK:
            m["xT8"] = np.ascontiguousarray(xT_c.astype(f8))
        in_maps.append(m)
    return in_maps


def kernel(**inputs):
    from concourse import bass_utils

    if "nc" not in _NC_CACHE:
        _NC_CACHE["nc"] = build_module()
    nc = _NC_CACHE["nc"]
    in_maps = _host_prep(inputs)
    res = bass_utils.run_bass_kernel_spmd(
        nc, in_maps, core_ids=list(range(NCORES))
    )
    outs = [r["out"].reshape(BPC, H, W, DIM) for r in res.results]
    return np.concatenate(outs, 0)


# revision 64
# speedup vs baseline: 1.5842x; 1.0039x over previous
"""Trainium2 Bass kernel for windowed attention with LoRA + decomposed rel-pos bias.

Full-input contract: kernel(**inputs) takes the unsharded numpy inputs and
returns the full (64, 14, 14, 768) float32 output.

Strategy (8 NeuronCores, data-parallel over the 64-window batch, 8 windows/core):
  Host prep (numpy):
    - Fold LoRA into qkv weights (exact); fold the 1/8 attention scale into
      Wq/bq; pre-transpose weights and x so every on-chip matmul contracts
      over SBUF partitions; gather rel-pos tables by the (q-k) index map.
    - q/k projection runs in fp8-e4m3 DoubleRow mode (two 128-row k-tiles
      per pass, 4x bf16 MACs): weights and x are scaled by 64 on the host
      (e4m3 min-normal vs w~0.02); the exact 1/4096 comes back out in the
      exp() activation scale. v and the output projection stay bf16 -- their
      quantization error would land on the output linearly, while q/k noise
      averages out across the softmax (measured end-to-end rel-err 1.7e-2
      vs the 2e-2 gate; bf16-only is 3.9e-3, set FP8_QK = 0 to fall back).
  On chip (per core, single fused pipeline, all SBUF resident):
    - one-hot / zero K-augmentation rows are built on-chip (GpSimd broadcast
      copies) or by three small DMA broadcasts (partition bases 78/14/46 are
      not engine-addressable); no big DMA broadcasts, no full-tile memset.
    - qk projection -> per-(window, head) "augmented" q/k tiles [128, 196]:
      rows hold q (or k) plus 14 rel-pos feature rows and 14 one-hot rows so
      ONE matmul per key-chunk produces q@k^T*scale + rel_h + rel_w directly
      in PSUM (K-augmentation trick). v projection groups are interleaved
      into the qk loop so its PSUM->SBUF copies hide under v's PE time.
    - rel features are computed in pumpable 4-row blocks interleaved between
      projection groups; copies rotate over ScalarE/VectorE (GPSIMD cannot
      access PSUM on real hardware).
    - key-token chunks 0:128 and 68:196 overlap so both exp activations
      cover 128 fully-valid partitions; attn@v contracts chunk0 rows 0:68
      and chunk1 rows 0:128 (disjoint tokens, partition bases 0).
    - one exp per (window, head-pair) over a 4-slot PSUM tile; softmax skips
      max-subtraction (logits are O(1)).
    - attn@v writes both parities into PSUM slot 0 (rows 0:64 / 64:128) and
      a 64-wide ones-block matmul broadcasts both softmax denominators onto
      the same rows in slot 1: one VectorE reciprocal + one multiply
      normalizes and writes o2 -- no cross-partition broadcast, no DRAM
      bounce (the hardware has no divide ALU op and TensorTensor may read
      only one PSUM operand, hence recip+mult through SBUF).
    - attention runs software-pipelined 2 deep (qk/exp of iteration b+2
      issue before attn@v of iteration b); all 1-bank PSUM tiles share one
      6-buffer pool so rotation never stalls TensorE.
    - output projection over 13 aligned 128-token chunks, bias added on
      VectorE, PSUM pool swapped per phase (6x1-bank -> 3x2-bank -> 2x1).
"""

import numpy as np
import ml_dtypes

B_TOTAL = 64
NCORES = 8
BPC = B_TOTAL // NCORES  # windows per core
H = W = 14
N = H * W  # 196 tokens per window
DIM = 768
NH = 12
HD = 64
DC = DIM // 128  # 6 contraction chunks
SCALE = HD ** -0.5  # 0.125, exact power of two

# key-token chunks: chunk0 = tokens 0:128, chunk1 = tokens 68:196 (overlap);
# attn@v contracts chunk1 rows 60:128 only (tokens 128:196).
CH1 = 68  # chunk1 token offset
AV0 = 68  # rows contracted from chunk0 (tokens 0:68); chunk1 covers 68:196

# row maps inside the 128-partition augmented q/k tiles
# even head parity: q/k rows 0:64, relh/kh-onehot 64:78, zeros 78:96,
#                   relw/kw-onehot 96:110; contraction range [0:110)
# odd  head parity: relw/kw-onehot 0:14, zeros 14:32, relh/kh-onehot 32:46,
#                   zeros 46:64, q/k rows 64:128; contraction range [0:128)
K_EVEN = 110
K_ODD = 128

VW = 66  # vall row width: [ones, v(64), ones]

# fp8 (e4m3) DoubleRow mode for the q/k projections: 0 = off (bf16),
# 1 = k only, 2 = q and k. Weights/x are pre-scaled by 64 on the host
# (e4m3 min-normal is 2^-6; w ~ 0.02); q/k land in SBUF scaled by 64 and
# the 1/4096 comes out exactly in the exp() scale. Measured end-to-end
# rel-err: off 3.9e-3, k8 1.13e-2, qk8 1.67e-2 (gate 2e-2).
FP8_QK = 2

_NC_CACHE = {}


def build_module(debug=False):
    from contextlib import ExitStack

    import concourse.tile as tile
    from concourse import bacc, mybir

    f32 = mybir.dt.float32
    bf16 = mybir.dt.bfloat16
    f8 = mybir.dt.float8e4
    PM = mybir.MatmulPerfMode
    AF = mybir.ActivationFunctionType
    ALU = mybir.AluOpType

    nc = bacc.Bacc(
        "TRN2", target_bir_lowering=False, debug=False, num_devices=NCORES
    )

    T = BPC * N  # 1568 tokens per core

    xT = nc.dram_tensor("xT", [DIM, T], bf16, kind="ExternalInput").ap()
    n8 = DIM * FP8_QK  # fp8 output-channel count (k only, or q and k)
    if FP8_QK:
        xT8 = nc.dram_tensor("xT8", [DIM, T], f8, kind="ExternalInput").ap()
        wqk8 = nc.dram_tensor("wqk8", [DIM, n8], f8, kind="ExternalInput").ap()
    if FP8_QK < 2:
        wqk = nc.dram_tensor(
            "wqk", [DIM, 2 * DIM - n8], bf16, kind="ExternalInput"
        ).ap()
    wv = nc.dram_tensor("wv", [DIM, DIM], bf16, kind="ExternalInput").ap()
    pw = nc.dram_tensor("pw", [DIM, DIM], bf16, kind="ExternalInput").ap()
    bqk = nc.dram_tensor("bqk", [2 * DIM], f32, kind="ExternalInput").ap()
    bv = nc.dram_tensor("bv", [DIM], bf16, kind="ExternalInput").ap()
    bp = nc.dram_tensor("bp", [DIM], bf16, kind="ExternalInput").ap()
    relh = nc.dram_tensor("relh", [HD, N], bf16, kind="ExternalInput").ap()
    relw = nc.dram_tensor("relw", [HD, N], bf16, kind="ExternalInput").ap()
    oh_e = nc.dram_tensor("oh_e", [46, N], bf16, kind="ExternalInput").ap()
    oh_o = nc.dram_tensor("oh_o", [64, N], bf16, kind="ExternalInput").ap()
    out = nc.dram_tensor("out", [T, DIM], f32, kind="ExternalOutput").ap()

    with tile.TileContext(nc) as tc, ExitStack() as ctx:
        singles = ctx.enter_context(tc.tile_pool(name="singles", bufs=1))
        po_pool = ctx.enter_context(tc.tile_pool(name="po", bufs=2, space="PSUM"))
        pa_cm = tc.tile_pool(name="pa", bufs=6, space="PSUM")
        pa_pool = pa_cm.__enter__()
        attn_pool = ctx.enter_context(tc.tile_pool(name="attn", bufs=8))
        osb_pool = ctx.enter_context(tc.tile_pool(name="osb", bufs=2))
        xt_pool_cm = tc.tile_pool(name="xt", bufs=1)
        xt_pool = xt_pool_cm.__enter__()

        # ---- resident SBUF tensors; DMA order = phase order ----
        # qk projection (fp8) runs first, v projection second, so its inputs
        # (wqk8/xT8/rel tables) load first and v's bf16 x streams in behind.
        bqk_sb = singles.tile([128, 2 * DC], f32)
        nc.sync.dma_start(out=bqk_sb[:], in_=bqk.rearrange("(c p) -> p c", p=128))
        ohe_sb = singles.tile([46, N], bf16)
        nc.sync.dma_start(out=ohe_sb[:], in_=oh_e)
        oho_sb = singles.tile([64, N], bf16)
        nc.sync.dma_start(out=oho_sb[:], in_=oh_o)
        if FP8_QK:
            wqk8_sb = singles.tile([128, DC, n8], f8)
            wqk8_r = wqk8.rearrange("(c p) o -> p c o", p=128)
            xT8_sb = xt_pool.tile([128, DC, T], f8)
            xT8_r = xT8.rearrange("(c p) t -> p c t", p=128)
            for c in range(DC):
                nc.sync.dma_start(out=wqk8_sb[:, c, :], in_=wqk8_r[:, c, :])
                nc.scalar.dma_start(
                    out=xT8_sb[:, c, 0 : 2 * N], in_=xT8_r[:, c, 0 : 2 * N]
                )
            for c in range(DC):
                nc.sync.dma_start(
                    out=xT8_sb[:, c, 2 * N :], in_=xT8_r[:, c, 2 * N :]
                )
        if FP8_QK < 2:
            wqk_sb = singles.tile([128, DC, 2 * DIM - n8], bf16)
            wqk_r = wqk.rearrange("(c p) o -> p c o", p=128)
            for c in range(DC):
                nc.sync.dma_start(out=wqk_sb[:, c, :], in_=wqk_r[:, c, :])
        relh_sb = singles.tile([128, N], bf16)
        nc.sync.dma_start(out=relh_sb[0:64, :], in_=relh)
        nc.sync.dma_start(out=relh_sb[64:128, :], in_=relh)
        relw_sb = singles.tile([128, N], bf16)
        nc.sync.dma_start(out=relw_sb[0:64, :], in_=relw)
        nc.sync.dma_start(out=relw_sb[64:128, :], in_=relw)
        bv_sb = singles.tile([128, DIM], bf16)
        nc.sync.dma_start(out=bv_sb[:], in_=bv.unsqueeze(0).broadcast_to([128, DIM]))
        wv_sb = singles.tile([128, DC, DIM], bf16)
        wv_r = wv.rearrange("(c p) o -> p c o", p=128)
        xT_sb = xt_pool.tile([128, DC, T], bf16)
        xT_r = xT.rearrange("(c p) t -> p c t", p=128)
        for c in range(DC):
            nc.sync.dma_start(out=wv_sb[:, c, :], in_=wv_r[:, c, :])
            nc.sync.dma_start(out=xT_sb[:, c, :], in_=xT_r[:, c, :])
        pw_sb = singles.tile([128, DC, DIM], bf16)
        nc.sync.dma_start(out=pw_sb[:], in_=pw.rearrange("(c p) o -> p c o", p=128))
        bp_sb = singles.tile([128, DIM], bf16)
        nc.sync.dma_start(out=bp_sb[:], in_=bp.unsqueeze(0).broadcast_to([128, DIM]))

        NPAIR = BPC * NH  # 96
        qaug = singles.tile([128, NPAIR, N], bf16)
        kaug = singles.tile([128, NPAIR, N], bf16)
        # [t-chunk partitions, window, chunk, head, ones+hd+ones]
        vall = singles.tile([128, BPC, 2, NH, VW], bf16)

        # ---- on-chip prep ----
        # one-hot K rows built by broadcast copies; q-side garbage bands are
        # zeroed by DMA broadcasts from oh_e's zero block (GPSIMD cannot
        # touch PSUM, and a full-tile memset would serialize ahead of the
        # projection copies). Bands are only read by attention (qk matmuls),
        # so these DMAs just need to land before then.
        ones64 = singles.tile([128, HD], bf16)
        nc.gpsimd.memset(ones64[:], 1.0)

        NPR = BPC * NH // 2  # 48 even/odd pair slots
        qv = qaug.rearrange(
            "p (b hh par) q -> p b hh par q", b=BPC, hh=NH // 2, par=2
        )
        qv6 = qaug.rearrange(
            "p (b hh par) (qh qw) -> p b hh par qh qw",
            b=BPC, hh=NH // 2, par=2, qh=H,
        )
        kpv = kaug.rearrange("p (pr par) q -> p pr par q", par=2)
        qpv = qaug.rearrange("p (pr par) q -> p pr par q", par=2)
        nc.gpsimd.tensor_copy(
            out=kpv[64:110, :, 0, :],
            in_=ohe_sb.rearrange("j q -> j () q").broadcast_to([46, NPR, N]),
        )
        nc.gpsimd.tensor_copy(
            out=kpv[0:64, :, 1, :],
            in_=oho_sb.rearrange("j q -> j () q").broadcast_to([64, NPR, N]),
        )
        zsrc = oh_e[14:32, :].rearrange("j q -> j () q")
        nc.sync.dma_start(out=qpv[78:96, :, 0, :], in_=zsrc.broadcast_to([18, NPR, N]))
        nc.sync.dma_start(out=qpv[14:32, :, 1, :], in_=zsrc.broadcast_to([18, NPR, N]))
        nc.sync.dma_start(out=qpv[46:64, :, 1, :], in_=zsrc.broadcast_to([18, NPR, N]))

        qp = qaug
        kp = kaug

        # ---- phase 1b: v projection groups (interleaved into phase 1) ----
        def emit_v_group(b, i, half):
            t0 = b * N + (0 if i == 0 else CH1)
            p_v = pa_pool.tile([128, 512], f32, tag="pa", name="p_v")
            for dc in range(DC):
                nc.tensor.matmul(
                    p_v[:, 0:384],
                    lhsT=xT_sb[:, dc, t0 : t0 + 128],
                    rhs=wv_sb[:, dc, half * 384 : (half + 1) * 384],
                    start=(dc == 0),
                    stop=(dc == DC - 1),
                )
            nc.vector.tensor_tensor(
                out=vall[:, b, i, 6 * half : 6 * half + 6, 1 : 1 + HD],
                in0=p_v[:, 0:384].rearrange("p (h d) -> p h d", h=6),
                in1=bv_sb[:, half * 384 : (half + 1) * 384].rearrange(
                    "p (h d) -> p h d", h=6
                ),
                op=ALU.add,
            )

        v_groups = [
            (b, i, half) for b in range(BPC) for i in range(2) for half in range(2)
        ]
        v_pos = [0]

        def emit_v(n):
            for _ in range(n):
                if v_pos[0] < len(v_groups):
                    emit_v_group(*v_groups[v_pos[0]])
                    v_pos[0] += 1



        # views for phase-1 destinations
        dest_v = [
            qaug.rearrange("p (b2 w2 h) q -> p b2 w2 h q", w2=2, h=NH),
            kaug.rearrange("p (b2 w2 h) q -> p b2 w2 h q", w2=2, h=NH),
        ]

        def emit_qkproj_group(oc, b2):
            is_q = oc < DC
            hh = (oc % DC) * 2
            dv = dest_v[0] if is_q else dest_v[1]
            p_qk = pa_pool.tile([128, 512], f32, tag="pa")
            use8 = FP8_QK == 2 or (FP8_QK == 1 and not is_q)
            if use8:
                oc8 = oc if FP8_QK == 2 else oc - DC
                for dr in range(DC // 2):
                    nc.tensor.matmul(
                        p_qk[:, 0 : 2 * N],
                        lhsT=wqk8_sb[
                            :, 2 * dr : 2 * dr + 2, oc8 * 128 : (oc8 + 1) * 128
                        ],
                        rhs=xT8_sb[:, 2 * dr : 2 * dr + 2, 2 * b2 * N : (2 * b2 + 2) * N],
                        start=(dr == 0),
                        stop=(dr == DC // 2 - 1),
                        perf_mode=PM.DoubleRow,
                    )
            else:
                oc_b = oc if FP8_QK == 0 else oc  # bf16 table holds q chunks only
                for dc in range(DC):
                    nc.tensor.matmul(
                        p_qk[:, 0 : 2 * N],
                        lhsT=wqk_sb[:, dc, oc_b * 128 : (oc_b + 1) * 128],
                        rhs=xT_sb[:, dc, 2 * b2 * N : (2 * b2 + 2) * N],
                        start=(dc == 0),
                        stop=(dc == DC - 1),
                    )
            nc.scalar.activation(
                out=dv[0:64, b2, :, hh, :],
                in_=p_qk[0:64, 0 : 2 * N].rearrange("p (w q) -> p w q", w=2),
                func=AF.Identity,
                bias=bqk_sb[0:64, oc : oc + 1],
                scale=1.0,
            )
            nc.vector.tensor_tensor(
                out=dv[64:128, b2, :, hh + 1, :],
                in0=p_qk[64:128, 0 : 2 * N].rearrange("p (w q) -> p w q", w=2),
                in1=bqk_sb[64:128, oc : oc + 1]
                .rearrange("p x -> p x ()")
                .broadcast_to([64, 2, N]),
                op=ALU.add,
            )

        def do_copy(ei, out, in_):
            if ei == 0:
                nc.vector.tensor_copy(out=out, in_=in_)
            elif ei == 1:
                nc.gpsimd.tensor_copy(out=out, in_=in_)
            else:
                nc.scalar.activation(out=out, in_=in_, func=AF.Copy, scale=1.0)

        # ---- rel features (phase 2), emitted as pumpable 4g blocks ----
        rel_eng = [2, 2, 0, 0]  # DVE/Act mix; GPSIMD cannot touch PSUM
        rel_ctr = [0]

        def make_rel_group(hx, blk, kind):
            par = hx % 2
            q_rows = slice(0, 64) if par == 0 else slice(64, 128)
            lh_base = 0 if par == 0 else 64
            relh_rows = slice(64, 78) if par == 0 else slice(32, 46)
            relw_rows = slice(96, 110) if par == 0 else slice(0, 14)
            relh_tp = (lh_base, 64 if par == 0 else 32)
            relw_tp = (lh_base, 96 if par == 0 else 0)
            hh, hp = hx // 2, hx % 2
            g0 = 4 * blk
            ng = min(4, H - g0)

            def emit():
                p_r = pa_pool.tile([128, 4, 128], f32, tag="pa", name="p_r")
                rows = relh_rows if kind == 0 else relw_rows
                for s in range(ng):
                    g = g0 + s
                    if kind == 0:
                        nc.tensor.matmul(
                            p_r[rows, s, 0 : BPC * W],
                            lhsT=relh_sb[q_rows, g * W : (g + 1) * W],
                            rhs=qv[q_rows, :, hh, hp, g * W : (g + 1) * W],
                            start=True,
                            stop=True,
                            tile_position=relh_tp,
                        )
                    else:
                        nc.tensor.matmul(
                            p_r[rows, s, 0 : BPC * W],
                            lhsT=relw_sb[q_rows, g * W : (g + 1) * W],
                            rhs=qv[q_rows, :, hh, hp, g : g + 13 * W + 1 : W],
                            start=True,
                            stop=True,
                            tile_position=relw_tp,
                        )
                ei = rel_eng[rel_ctr[0] % len(rel_eng)]
                rel_ctr[0] += 1
                if kind == 0:
                    do_copy(
                        ei,
                        qv6[rows, :, hh, hp, g0 : g0 + ng, :],
                        p_r[rows, 0:ng, 0 : BPC * W].rearrange(
                            "p s (b w) -> p b s w", b=BPC
                        ),
                    )
                else:
                    do_copy(
                        ei,
                        qv6[rows, :, hh, hp, :, g0 : g0 + ng],
                        p_r[rows, 0:ng, 0 : BPC * W].rearrange(
                            "p s (b q) -> p b q s", b=BPC
                        ),
                    )

            return emit

        pending_rel = []

        def pump(n):
            for _ in range(min(n, len(pending_rel))):
                pending_rel.pop(0)()

        # ---- attention inner stages ----
        def emit_qk_exp(b, hh, a_sb):
            p_a = pb_pool.tile([128, 4, 256], f32, tag="pab")
            for par in range(2):
                hx = 2 * hh + par
                pair = b * NH + hx
                krange = slice(0, K_EVEN) if par == 0 else slice(0, K_ODD)
                nc.tensor.matmul(
                    p_a[:, 2 * par, 0:N],
                    lhsT=kp[krange, pair, 0:128],
                    rhs=qp[krange, pair, :],
                    start=True,
                    stop=True,
                )
                nc.tensor.matmul(
                    p_a[:, 2 * par + 1, 0:N],
                    lhsT=kp[krange, pair, CH1:N],
                    rhs=qp[krange, pair, :],
                    start=True,
                    stop=True,
                )
            nc.scalar.activation(
                out=a_sb[:],
                in_=p_a[:, :, 0:N],
                func=AF.Exp,
                scale=1.0 / (64.0 ** FP8_QK),
            )

        def emit_av(b, hh, a_sb, p_o):
            h0, h1 = 2 * hh, 2 * hh + 1
            # slot 0: attn@v for both parities (rows 0:64 even, 64:128 odd);
            # slot 1: softmax denominators broadcast to the same rows via a
            # 64-wide ones block, so ONE divide normalizes and writes o2.
            for par, h in ((0, h0), (1, h1)):
                tp = (0, 0) if par == 0 else (0, 64)
                rows = slice(0, 64) if par == 0 else slice(64, 128)
                nc.tensor.matmul(
                    p_o[rows, 0, 0:N],
                    lhsT=vall[0:AV0, b, 0, h, 1 : 1 + HD],
                    rhs=a_sb[0:AV0, 2 * par, :],
                    start=True,
                    stop=False,
                    tile_position=tp,
                    skip_group_check=True,
                )
                nc.tensor.matmul(
                    p_o[rows, 0, 0:N],
                    lhsT=vall[:, b, 1, h, 1 : 1 + HD],
                    rhs=a_sb[:, 2 * par + 1, :],
                    start=False,
                    stop=True,
                    tile_position=tp,
                    skip_group_check=True,
                )
                nc.tensor.matmul(
                    p_o[rows, 1, 0:N],
                    lhsT=ones64[0:AV0, :],
                    rhs=a_sb[0:AV0, 2 * par, :],
                    start=True,
                    stop=False,
                    tile_position=tp,
                    skip_group_check=True,
                )
                nc.tensor.matmul(
                    p_o[rows, 1, 0:N],
                    lhsT=ones64[:],
                    rhs=a_sb[:, 2 * par + 1, :],
                    start=False,
                    stop=True,
                    tile_position=tp,
                    skip_group_check=True,
                )

        # ---- phase 1: qk + v projections interleaved, rel pumped in ----
        gctr = 0
        for hh in range(NH // 2):
            for oc in (hh, DC + hh):
                for b2 in range(BPC // 2):
                    emit_qkproj_group(oc, b2)
                    pump(2)
                    gctr += 1
                    if gctr > 16:
                        emit_v((gctr - 16) - v_pos[0])
                if oc < DC and b2 == BPC // 2 - 1:
                    for hx in (2 * hh, 2 * hh + 1):
                        for blk in range(4):
                            for kind in range(2):
                                pending_rel.append(make_rel_group(hx, blk, kind))

        # drain remaining rel groups and v
        pump(len(pending_rel))
        emit_v(len(v_groups))
        pa_cm.__exit__(None, None, None)
        pb_cm = tc.tile_pool(name="pb", bufs=3, space="PSUM")
        pb_pool = pb_cm.__enter__()

        # xT no longer needed; free its zone for the persistent o2 tile
        xt_pool_cm.__exit__(None, None, None)
        o2_pool = ctx.enter_context(tc.tile_pool(name="o2", bufs=1))
        o2_all = o2_pool.tile([128, DC, T], bf16)

        # ---- phase 3: attention, per head pair ----
        for hh in range(NH // 2):
            inflight = []

            def drain_one():
                pb, xa = inflight.pop(0)
                p_o = po_pool.tile([128, 2, 256], f32, tag="po")
                emit_av(pb, hh, xa, p_o)
                # hw: no divide ALU op, and TensorTensor reads at most one
                # input from PSUM -> reciprocal to SBUF, then multiply
                den_sb = attn_pool.tile([128, N], f32, tag="den")
                nc.vector.reciprocal(out=den_sb[:], in_=p_o[:, 1, 0:N])
                # (recip stays on DVE: Act is exp-saturated in attention)
                with nc.allow_low_precision(reason="bf16 softmax"):
                    nc.vector.tensor_tensor(
                        out=o2_all[:, hh, pb * N : (pb + 1) * N],
                        in0=p_o[:, 0, 0:N],
                        in1=den_sb[:],
                        op=ALU.mult,
                    )

            for b in range(BPC):
                a_sb = attn_pool.tile([128, 4, N], bf16, tag="a0")
                emit_qk_exp(b, hh, a_sb)
                pump(2)
                inflight.append((b, a_sb))
                if len(inflight) > 2:
                    drain_one()
            while inflight:
                drain_one()

        # ---- phase 4: projection over global 128-token chunks ----
        pb_cm.__exit__(None, None, None)
        ps4_cm = tc.tile_pool(name="ps4", bufs=2, space="PSUM")
        ps4 = ps4_cm.__enter__()
        NT_CH = (T + 127) // 128  # 13
        for j in range(NT_CH):
            t0 = j * 128
            tc_rows = min(128, T - t0)
            o_sb = osb_pool.tile([128, DIM], f32, tag="osb")
            for half in range(2):
                p_p = ps4.tile([128, 512], f32, tag="ps")
                for cc in range(DC):
                    nc.tensor.matmul(
                        p_p[0:tc_rows, 0:384],
                        lhsT=o2_all[:, cc, t0 : t0 + tc_rows],
                        rhs=pw_sb[:, cc, half * 384 : (half + 1) * 384],
                        start=(cc == 0),
                        stop=(cc == DC - 1),
                    )
                nc.vector.tensor_tensor(
                    out=o_sb[0:tc_rows, half * 384 : (half + 1) * 384],
                    in0=p_p[0:tc_rows, 0:384],
                    in1=bp_sb[0:tc_rows, half * 384 : (half + 1) * 384],
                    op=ALU.add,
                )
                nc.sync.dma_start(
                    out=out[t0 : t0 + tc_rows, half * 384 : (half + 1) * 384],
                    in_=o_sb[0:tc_rows, half * 384 : (half + 1) * 384],
                )
        ps4_cm.__exit__(None, None, None)

    nc.finalize()
    return nc


def _host_prep(inputs):
    bf16 = ml_dtypes.bfloat16
    f8 = ml_dtypes.float8_e4m3fn
    x = np.asarray(inputs["x"], np.float32)
    qkv_w = np.asarray(inputs["qkv_w"], np.float32)
    qkv_b = np.asarray(inputs["qkv_b"], np.float32)
    proj_w = np.asarray(inputs["proj_w"], np.float32)
    proj_b = np.asarray(inputs["proj_b"], np.float32)
    la_q = np.asarray(inputs["la_q"], np.float32)
    lb_q = np.asarray(inputs["lb_q"], np.float32)
    la_v = np.asarray(inputs["la_v"], np.float32)
    lb_v = np.asarray(inputs["lb_v"], np.float32)
    rel_pos_h = np.asarray(inputs["rel_pos_h"], np.float32)
    rel_pos_w = np.asarray(inputs["rel_pos_w"], np.float32)

    Wq = qkv_w[:DIM] + lb_q @ la_q
    Wk = qkv_w[DIM : 2 * DIM]
    Wv = qkv_w[2 * DIM :] + lb_v @ la_v

    s8 = 64.0
    if FP8_QK == 2:
        wqk8_host = np.ascontiguousarray(
            np.concatenate([s8 * SCALE * Wq, s8 * Wk], 0).T.astype(f8)
        )
        bqk_host = (
            s8 * np.concatenate([SCALE * qkv_b[:DIM], qkv_b[DIM : 2 * DIM]])
        ).astype(np.float32)
    elif FP8_QK == 1:
        wqk8_host = np.ascontiguousarray((s8 * Wk).T.astype(f8))
        wqk_host = np.ascontiguousarray((SCALE * Wq).T.astype(bf16))
        bqk_host = np.concatenate(
            [SCALE * qkv_b[:DIM], s8 * qkv_b[DIM : 2 * DIM]]
        ).astype(np.float32)
    else:
        wqk_host = np.ascontiguousarray(
            np.concatenate([SCALE * Wq, Wk], 0).T.astype(bf16)
        )
        bqk_host = np.concatenate(
            [SCALE * qkv_b[:DIM], qkv_b[DIM : 2 * DIM]]
        ).astype(np.float32)
    wv_host = np.ascontiguousarray(Wv.T.astype(bf16))
    pw_host = np.ascontiguousarray(proj_w.T.astype(bf16))
    bv_host = np.ascontiguousarray(qkv_b[2 * DIM :].astype(bf16))
    bp_host = np.ascontiguousarray(proj_b.astype(bf16))

    idx = np.arange(H)[:, None] - np.arange(H)[None, :] + (H - 1)
    Rh = rel_pos_h[idx]  # [qh, kh_j, hd]
    Rw = rel_pos_w[idx]  # [qw, kw_j, hd]
    rscale = s8 if FP8_QK else 1.0
    relh_host = np.ascontiguousarray(
        (rscale * Rh / SCALE).transpose(2, 0, 1).reshape(HD, N).astype(bf16)
    )
    relw_host = np.ascontiguousarray(
        (rscale * Rw / SCALE).transpose(2, 0, 1).reshape(HD, N).astype(bf16)
    )

    kt = np.arange(N)
    oh_kh = (kt[None, :] // W == np.arange(H)[:, None]).astype(bf16)  # [14, 196]
    oh_kw = (kt[None, :] % W == np.arange(W)[:, None]).astype(bf16)
    z18 = np.zeros((18, N), bf16)
    oh_e_host = np.ascontiguousarray(np.concatenate([oh_kh, z18, oh_kw], 0))
    oh_o_host = np.ascontiguousarray(
        np.concatenate([oh_kw, z18, oh_kh, z18], 0)
    )

    shared = {
        "wv": wv_host,
        "pw": pw_host,
        "bqk": bqk_host,
        "bv": bv_host,
        "bp": bp_host,
        "relh": relh_host,
        "relw": relw_host,
        "oh_e": oh_e_host,
        "oh_o": oh_o_host,
    }
    if FP8_QK:
        shared["wqk8"] = wqk8_host
    if FP8_QK < 2:
        shared["wqk"] = wqk_host

    x_flat = x.reshape(B_TOTAL, N, DIM)
    in_maps = []
    for c in range(NCORES):
        xc = x_flat[c * BPC : (c + 1) * BPC].reshape(BPC * N, DIM)
        xT_c = np.ascontiguousarray(xc.T.astype(bf16))
        m = dict(shared)
        m["xT"] = xT_c
        if FP8_QK:
            m["xT8"] = np.ascontiguousarray(xT_c.astype(f8))
        in_maps.append(m)
    return in_maps


def kernel(**inputs):
    from concourse import bass_utils

    if "nc" not in _NC_CACHE:
        _NC_CACHE["nc"] = build_module()
    nc = _NC_CACHE["nc"]
    in_maps = _host_prep(inputs)
    res = bass_utils.run_bass_kernel_spmd(
        nc, in_maps, core_ids=list(range(NCORES))
    )
    outs = [r["out"].reshape(BPC, H, W, DIM) for r in res.results]
    return np.concatenate(outs, 0)


# revision 67
# speedup vs baseline: 1.5981x; 1.0087x over previous
"""Trainium2 Bass kernel for windowed attention with LoRA + decomposed rel-pos bias.

Full-input contract: kernel(**inputs) takes the unsharded numpy inputs and
returns the full (64, 14, 14, 768) float32 output.

Strategy (8 NeuronCores, data-parallel over the 64-window batch, 8 windows/core):
  Host prep (numpy):
    - Fold LoRA into qkv weights (exact); fold the 1/8 attention scale into
      Wq/bq; pre-transpose weights and x so every on-chip matmul contracts
      over SBUF partitions; gather rel-pos tables by the (q-k) index map.
    - q/k projection runs in fp8-e4m3 DoubleRow mode (two 128-row k-tiles
      per pass, 4x bf16 MACs): weights and x are scaled by 64 on the host
      (e4m3 min-normal vs w~0.02); the exact 1/4096 comes back out in the
      exp() activation scale. v and the output projection stay bf16 -- their
      quantization error would land on the output linearly, while q/k noise
      averages out across the softmax (measured end-to-end rel-err 1.7e-2
      vs the 2e-2 gate; bf16-only is 3.9e-3, set FP8_QK = 0 to fall back).
  On chip (per core, single fused pipeline, all SBUF resident):
    - one-hot / zero K-augmentation rows are built on-chip (GpSimd broadcast
      copies) or by three small DMA broadcasts (partition bases 78/14/46 are
      not engine-addressable); no big DMA broadcasts, no full-tile memset.
    - qk projection -> per-(window, head) "augmented" q/k tiles [128, 196]:
      rows hold q (or k) plus 14 rel-pos feature rows and 14 one-hot rows so
      ONE matmul per key-chunk produces q@k^T*scale + rel_h + rel_w directly
      in PSUM (K-augmentation trick). v projection groups are interleaved
      into the qk loop so its PSUM->SBUF copies hide under v's PE time.
    - rel features are computed in pumpable 4-row blocks interleaved between
      projection groups; copies rotate over ScalarE/VectorE (GPSIMD cannot
      access PSUM on real hardware).
    - key-token chunks 0:128 and 68:196 overlap so both exp activations
      cover 128 fully-valid partitions; attn@v contracts chunk0 rows 0:68
      and chunk1 rows 0:128 (disjoint tokens, partition bases 0).
    - one exp per (window, head-pair) over a 4-slot PSUM tile; softmax skips
      max-subtraction (logits are O(1)).
    - attn@v writes both parities into PSUM slot 0 (rows 0:64 / 64:128) and
      a 64-wide ones-block matmul broadcasts both softmax denominators onto
      the same rows in slot 1: one VectorE reciprocal + one multiply
      normalizes and writes o2 -- no cross-partition broadcast, no DRAM
      bounce (the hardware has no divide ALU op and TensorTensor may read
      only one PSUM operand, hence recip+mult through SBUF).
    - attention runs software-pipelined 2 deep (qk/exp of iteration b+2
      issue before attn@v of iteration b); all 1-bank PSUM tiles share one
      6-buffer pool so rotation never stalls TensorE.
    - output projection over 13 aligned 128-token chunks, bias added on
      VectorE, PSUM pool swapped per phase (6x1-bank -> 3x2-bank -> 2x1).
"""

import numpy as np
import ml_dtypes

B_TOTAL = 64
NCORES = 8
BPC = B_TOTAL // NCORES  # windows per core
H = W = 14
N = H * W  # 196 tokens per window
DIM = 768
NH = 12
HD = 64
DC = DIM // 128  # 6 contraction chunks
SCALE = HD ** -0.5  # 0.125, exact power of two

# key-token chunks: chunk0 = tokens 0:128, chunk1 = tokens 68:196 (overlap);
# attn@v contracts chunk1 rows 60:128 only (tokens 128:196).
CH1 = 68  # chunk1 token offset
AV0 = 68  # rows contracted from chunk0 (tokens 0:68); chunk1 covers 68:196

# row maps inside the 128-partition augmented q/k tiles
# even head parity: q/k rows 0:64, relh/kh-onehot 64:78, zeros 78:96,
#                   relw/kw-onehot 96:110; contraction range [0:110)
# odd  head parity: relw/kw-onehot 0:14, zeros 14:32, relh/kh-onehot 32:46,
#                   zeros 46:64, q/k rows 64:128; contraction range [0:128)
K_EVEN = 110
K_ODD = 128

VW = 66  # vall row width: [ones, v(64), ones]

# fp8 (e4m3) DoubleRow mode for the q/k projections: 0 = off (bf16),
# 1 = k only, 2 = q and k. Weights/x are pre-scaled by 64 on the host
# (e4m3 min-normal is 2^-6; w ~ 0.02); q/k land in SBUF scaled by 64 and
# the 1/4096 comes out exactly in the exp() scale. Measured end-to-end
# rel-err: off 3.9e-3, k8 1.13e-2, qk8 1.67e-2 (gate 2e-2).
FP8_QK = 2

_NC_CACHE = {}


def build_module(debug=False):
    from contextlib import ExitStack

    import concourse.tile as tile
    from concourse import bacc, mybir

    f32 = mybir.dt.float32
    bf16 = mybir.dt.bfloat16
    f8 = mybir.dt.float8e4
    PM = mybir.MatmulPerfMode
    AF = mybir.ActivationFunctionType
    ALU = mybir.AluOpType

    nc = bacc.Bacc(
        "TRN2", target_bir_lowering=False, debug=False, num_devices=NCORES
    )

    T = BPC * N  # 1568 tokens per core

    xT = nc.dram_tensor("xT", [DIM, T], bf16, kind="ExternalInput").ap()
    n8 = DIM * FP8_QK  # fp8 output-channel count (k only, or q and k)
    if FP8_QK:
        xT8 = nc.dram_tensor("xT8", [DIM, T], f8, kind="ExternalInput").ap()
        wqk8 = nc.dram_tensor("wqk8", [DIM, n8], f8, kind="ExternalInput").ap()
    if FP8_QK < 2:
        wqk = nc.dram_tensor(
            "wqk", [DIM, 2 * DIM - n8], bf16, kind="ExternalInput"
        ).ap()
    wv = nc.dram_tensor("wv", [DIM, DIM], bf16, kind="ExternalInput").ap()
    pw = nc.dram_tensor("pw", [DIM, DIM], bf16, kind="ExternalInput").ap()
    bqk = nc.dram_tensor("bqk", [2 * DIM], f32, kind="ExternalInput").ap()
    bv = nc.dram_tensor("bv", [DIM], bf16, kind="ExternalInput").ap()
    bp = nc.dram_tensor("bp", [DIM], bf16, kind="ExternalInput").ap()
    relh = nc.dram_tensor("relh", [HD, N], bf16, kind="ExternalInput").ap()
    relw = nc.dram_tensor("relw", [HD, N], bf16, kind="ExternalInput").ap()
    oh_e = nc.dram_tensor("oh_e", [46, N], bf16, kind="ExternalInput").ap()
    oh_o = nc.dram_tensor("oh_o", [64, N], bf16, kind="ExternalInput").ap()
    out = nc.dram_tensor("out", [T, DIM], f32, kind="ExternalOutput").ap()

    with tile.TileContext(nc) as tc, ExitStack() as ctx:
        singles = ctx.enter_context(tc.tile_pool(name="singles", bufs=1))
        po_pool = ctx.enter_context(tc.tile_pool(name="po", bufs=2, space="PSUM"))
        pa_cm = tc.tile_pool(name="pa", bufs=6, space="PSUM")
        pa_pool = pa_cm.__enter__()
        attn_pool = ctx.enter_context(tc.tile_pool(name="attn", bufs=8))
        osb_pool = ctx.enter_context(tc.tile_pool(name="osb", bufs=2))
        xt_pool_cm = tc.tile_pool(name="xt", bufs=1)
        xt_pool = xt_pool_cm.__enter__()

        # ---- resident SBUF tensors; DMA order = phase order ----
        # qk projection (fp8) runs first, v projection second, so its inputs
        # (wqk8/xT8/rel tables) load first and v's bf16 x streams in behind.
        bqk_sb = singles.tile([128, 2 * DC], f32)
        nc.sync.dma_start(out=bqk_sb[:], in_=bqk.rearrange("(c p) -> p c", p=128))
        ohe_sb = singles.tile([46, N], bf16)
        oho_sb = singles.tile([64, N], bf16)
        if FP8_QK:
            wqk8_sb = singles.tile([128, DC, n8], f8)
            wqk8_r = wqk8.rearrange("(c p) o -> p c o", p=128)
            xT8_sb = xt_pool.tile([128, DC, T], f8)
            xT8_r = xT8.rearrange("(c p) t -> p c t", p=128)
            for c in range(DC):
                nc.sync.dma_start(out=wqk8_sb[:, c, :], in_=wqk8_r[:, c, :])
                nc.scalar.dma_start(
                    out=xT8_sb[:, c, 0 : 2 * N], in_=xT8_r[:, c, 0 : 2 * N]
                )
            for c in range(DC):
                nc.sync.dma_start(
                    out=xT8_sb[:, c, 2 * N :], in_=xT8_r[:, c, 2 * N :]
                )
        if FP8_QK < 2:
            wqk_sb = singles.tile([128, DC, 2 * DIM - n8], bf16)
            wqk_r = wqk.rearrange("(c p) o -> p c o", p=128)
            for c in range(DC):
                nc.sync.dma_start(out=wqk_sb[:, c, :], in_=wqk_r[:, c, :])
        nc.sync.dma_start(out=ohe_sb[:], in_=oh_e)
        nc.sync.dma_start(out=oho_sb[:], in_=oh_o)
        relh_sb = singles.tile([128, N], bf16)
        nc.sync.dma_start(out=relh_sb[0:64, :], in_=relh)
        nc.sync.dma_start(out=relh_sb[64:128, :], in_=relh)
        relw_sb = singles.tile([128, N], bf16)
        nc.sync.dma_start(out=relw_sb[0:64, :], in_=relw)
        nc.sync.dma_start(out=relw_sb[64:128, :], in_=relw)
        bv_sb = singles.tile([128, DIM], bf16)
        nc.sync.dma_start(out=bv_sb[:], in_=bv.unsqueeze(0).broadcast_to([128, DIM]))
        wv_sb = singles.tile([128, DC, DIM], bf16)
        wv_r = wv.rearrange("(c p) o -> p c o", p=128)
        xT_sb = xt_pool.tile([128, DC, T], bf16)
        xT_r = xT.rearrange("(c p) t -> p c t", p=128)
        for c in range(DC):
            nc.sync.dma_start(out=wv_sb[:, c, :], in_=wv_r[:, c, :])
            nc.sync.dma_start(out=xT_sb[:, c, :], in_=xT_r[:, c, :])
        pw_sb = singles.tile([128, DC, DIM], bf16)
        nc.sync.dma_start(out=pw_sb[:], in_=pw.rearrange("(c p) o -> p c o", p=128))
        bp_sb = singles.tile([128, DIM], bf16)
        nc.sync.dma_start(out=bp_sb[:], in_=bp.unsqueeze(0).broadcast_to([128, DIM]))

        NPAIR = BPC * NH  # 96
        qaug = singles.tile([128, NPAIR, N], bf16)
        kaug = singles.tile([128, NPAIR, N], bf16)
        # [t-chunk partitions, window, chunk, head, ones+hd+ones]
        vall = singles.tile([128, BPC, 2, NH, VW], bf16)

        # ---- on-chip prep ----
        # one-hot K rows built by broadcast copies; q-side garbage bands are
        # zeroed by DMA broadcasts from oh_e's zero block (GPSIMD cannot
        # touch PSUM, and a full-tile memset would serialize ahead of the
        # projection copies). Bands are only read by attention (qk matmuls),
        # so these DMAs just need to land before then.
        ones64 = singles.tile([128, HD], bf16)
        nc.gpsimd.memset(ones64[:], 1.0)

        NPR = BPC * NH // 2  # 48 even/odd pair slots
        qv = qaug.rearrange(
            "p (b hh par) q -> p b hh par q", b=BPC, hh=NH // 2, par=2
        )
        qv6 = qaug.rearrange(
            "p (b hh par) (qh qw) -> p b hh par qh qw",
            b=BPC, hh=NH // 2, par=2, qh=H,
        )
        kpv = kaug.rearrange("p (pr par) q -> p pr par q", par=2)
        qpv = qaug.rearrange("p (pr par) q -> p pr par q", par=2)
        nc.gpsimd.tensor_copy(
            out=kpv[64:110, :, 0, :],
            in_=ohe_sb.rearrange("j q -> j () q").broadcast_to([46, NPR, N]),
        )
        nc.gpsimd.tensor_copy(
            out=kpv[0:64, :, 1, :],
            in_=oho_sb.rearrange("j q -> j () q").broadcast_to([64, NPR, N]),
        )
        zsrc = oh_e[14:32, :].rearrange("j q -> j () q")
        nc.sync.dma_start(out=qpv[78:96, :, 0, :], in_=zsrc.broadcast_to([18, NPR, N]))
        nc.sync.dma_start(out=qpv[14:32, :, 1, :], in_=zsrc.broadcast_to([18, NPR, N]))
        nc.sync.dma_start(out=qpv[46:64, :, 1, :], in_=zsrc.broadcast_to([18, NPR, N]))

        qp = qaug
        kp = kaug

        # ---- phase 1b: v projection groups (interleaved into phase 1) ----
        def emit_v_group(b, i, half):
            t0 = b * N + (0 if i == 0 else CH1)
            p_v = pa_pool.tile([128, 512], f32, tag="pa", name="p_v")
            for dc in range(DC):
                nc.tensor.matmul(
                    p_v[:, 0:384],
                    lhsT=xT_sb[:, dc, t0 : t0 + 128],
                    rhs=wv_sb[:, dc, half * 384 : (half + 1) * 384],
                    start=(dc == 0),
                    stop=(dc == DC - 1),
                )
            nc.vector.tensor_tensor(
                out=vall[:, b, i, 6 * half : 6 * half + 6, 1 : 1 + HD],
                in0=p_v[:, 0:384].rearrange("p (h d) -> p h d", h=6),
                in1=bv_sb[:, half * 384 : (half + 1) * 384].rearrange(
                    "p (h d) -> p h d", h=6
                ),
                op=ALU.add,
            )

        v_groups = [
            (b, i, half) for b in range(BPC) for i in range(2) for half in range(2)
        ]
        v_pos = [0]

        def emit_v(n):
            for _ in range(n):
                if v_pos[0] < len(v_groups):
                    emit_v_group(*v_groups[v_pos[0]])
                    v_pos[0] += 1



        # views for phase-1 destinations
        dest_v = [
            qaug.rearrange("p (b2 w2 h) q -> p b2 w2 h q", w2=2, h=NH),
            kaug.rearrange("p (b2 w2 h) q -> p b2 w2 h q", w2=2, h=NH),
        ]

        def emit_qkproj_group(oc, b2):
            is_q = oc < DC
            hh = (oc % DC) * 2
            dv = dest_v[0] if is_q else dest_v[1]
            p_qk = pa_pool.tile([128, 512], f32, tag="pa")
            use8 = FP8_QK == 2 or (FP8_QK == 1 and not is_q)
            if use8:
                oc8 = oc if FP8_QK == 2 else oc - DC
                for dr in range(DC // 2):
                    nc.tensor.matmul(
                        p_qk[:, 0 : 2 * N],
                        lhsT=wqk8_sb[
                            :, 2 * dr : 2 * dr + 2, oc8 * 128 : (oc8 + 1) * 128
                        ],
                        rhs=xT8_sb[:, 2 * dr : 2 * dr + 2, 2 * b2 * N : (2 * b2 + 2) * N],
                        start=(dr == 0),
                        stop=(dr == DC // 2 - 1),
                        perf_mode=PM.DoubleRow,
                    )
            else:
                oc_b = oc if FP8_QK == 0 else oc  # bf16 table holds q chunks only
                for dc in range(DC):
                    nc.tensor.matmul(
                        p_qk[:, 0 : 2 * N],
                        lhsT=wqk_sb[:, dc, oc_b * 128 : (oc_b + 1) * 128],
                        rhs=xT_sb[:, dc, 2 * b2 * N : (2 * b2 + 2) * N],
                        start=(dc == 0),
                        stop=(dc == DC - 1),
                    )
            nc.scalar.activation(
                out=dv[0:64, b2, :, hh, :],
                in_=p_qk[0:64, 0 : 2 * N].rearrange("p (w q) -> p w q", w=2),
                func=AF.Identity,
                bias=bqk_sb[0:64, oc : oc + 1],
                scale=1.0,
            )
            nc.vector.tensor_tensor(
                out=dv[64:128, b2, :, hh + 1, :],
                in0=p_qk[64:128, 0 : 2 * N].rearrange("p (w q) -> p w q", w=2),
                in1=bqk_sb[64:128, oc : oc + 1]
                .rearrange("p x -> p x ()")
                .broadcast_to([64, 2, N]),
                op=ALU.add,
            )

        def do_copy(ei, out, in_):
            if ei == 0:
                nc.vector.tensor_copy(out=out, in_=in_)
            elif ei == 1:
                nc.gpsimd.tensor_copy(out=out, in_=in_)
            else:
                nc.scalar.activation(out=out, in_=in_, func=AF.Copy, scale=1.0)

        # ---- rel features (phase 2), emitted as pumpable 4g blocks ----
        rel_eng = [2, 2, 0, 0]  # DVE/Act mix; GPSIMD cannot touch PSUM
        rel_ctr = [0]

        def make_rel_group(hx, blk, kind):
            par = hx % 2
            q_rows = slice(0, 64) if par == 0 else slice(64, 128)
            lh_base = 0 if par == 0 else 64
            relh_rows = slice(64, 78) if par == 0 else slice(32, 46)
            relw_rows = slice(96, 110) if par == 0 else slice(0, 14)
            relh_tp = (lh_base, 64 if par == 0 else 32)
            relw_tp = (lh_base, 96 if par == 0 else 0)
            hh, hp = hx // 2, hx % 2
            g0 = 4 * blk
            ng = min(4, H - g0)

            def emit():
                p_r = pa_pool.tile([128, 4, 128], f32, tag="pa", name="p_r")
                rows = relh_rows if kind == 0 else relw_rows
                for s in range(ng):
                    g = g0 + s
                    if kind == 0:
                        nc.tensor.matmul(
                            p_r[rows, s, 0 : BPC * W],
                            lhsT=relh_sb[q_rows, g * W : (g + 1) * W],
                            rhs=qv[q_rows, :, hh, hp, g * W : (g + 1) * W],
                            start=True,
                            stop=True,
                            tile_position=relh_tp,
                        )
                    else:
                        nc.tensor.matmul(
                            p_r[rows, s, 0 : BPC * W],
                            lhsT=relw_sb[q_rows, g * W : (g + 1) * W],
                            rhs=qv[q_rows, :, hh, hp, g : g + 13 * W + 1 : W],
                            start=True,
                            stop=True,
                            tile_position=relw_tp,
                        )
                ei = rel_eng[rel_ctr[0] % len(rel_eng)]
                rel_ctr[0] += 1
                if kind == 0:
                    do_copy(
                        ei,
                        qv6[rows, :, hh, hp, g0 : g0 + ng, :],
                        p_r[rows, 0:ng, 0 : BPC * W].rearrange(
                            "p s (b w) -> p b s w", b=BPC
                        ),
                    )
                else:
                    do_copy(
                        ei,
                        qv6[rows, :, hh, hp, :, g0 : g0 + ng],
                        p_r[rows, 0:ng, 0 : BPC * W].rearrange(
                            "p s (b q) -> p b q s", b=BPC
                        ),
                    )

            return emit

        pending_rel = []

        def pump(n):
            for _ in range(min(n, len(pending_rel))):
                pending_rel.pop(0)()

        # ---- attention inner stages ----
        def emit_qk_exp(b, hh, a_sb):
            p_a = pb_pool.tile([128, 4, 256], f32, tag="pab")
            for par in range(2):
                hx = 2 * hh + par
                pair = b * NH + hx
                krange = slice(0, K_EVEN) if par == 0 else slice(0, K_ODD)
                nc.tensor.matmul(
                    p_a[:, 2 * par, 0:N],
                    lhsT=kp[krange, pair, 0:128],
                    rhs=qp[krange, pair, :],
                    start=True,
                    stop=True,
                )
                nc.tensor.matmul(
                    p_a[:, 2 * par + 1, 0:N],
                    lhsT=kp[krange, pair, CH1:N],
                    rhs=qp[krange, pair, :],
                    start=True,
                    stop=True,
                )
            nc.scalar.activation(
                out=a_sb[:],
                in_=p_a[:, :, 0:N],
                func=AF.Exp,
                scale=1.0 / (64.0 ** FP8_QK),
            )

        def emit_av(b, hh, a_sb, p_o):
            h0, h1 = 2 * hh, 2 * hh + 1
            # slot 0: attn@v for both parities (rows 0:64 even, 64:128 odd);
            # slot 1: softmax denominators broadcast to the same rows via a
            # 64-wide ones block, so ONE divide normalizes and writes o2.
            for par, h in ((0, h0), (1, h1)):
                tp = (0, 0) if par == 0 else (0, 64)
                rows = slice(0, 64) if par == 0 else slice(64, 128)
                nc.tensor.matmul(
                    p_o[rows, 0, 0:N],
                    lhsT=vall[0:AV0, b, 0, h, 1 : 1 + HD],
                    rhs=a_sb[0:AV0, 2 * par, :],
                    start=True,
                    stop=False,
                    tile_position=tp,
                    skip_group_check=True,
                )
                nc.tensor.matmul(
                    p_o[rows, 0, 0:N],
                    lhsT=vall[:, b, 1, h, 1 : 1 + HD],
                    rhs=a_sb[:, 2 * par + 1, :],
                    start=False,
                    stop=True,
                    tile_position=tp,
                    skip_group_check=True,
                )
                nc.tensor.matmul(
                    p_o[rows, 1, 0:N],
                    lhsT=ones64[0:AV0, :],
                    rhs=a_sb[0:AV0, 2 * par, :],
                    start=True,
                    stop=False,
                    tile_position=tp,
                    skip_group_check=True,
                )
                nc.tensor.matmul(
                    p_o[rows, 1, 0:N],
                    lhsT=ones64[:],
                    rhs=a_sb[:, 2 * par + 1, :],
                    start=False,
                    stop=True,
                    tile_position=tp,
                    skip_group_check=True,
                )

        # ---- phase 1: qk + v projections interleaved, rel pumped in ----
        gctr = 0
        for hh in range(NH // 2):
            for oc in (hh, DC + hh):
                for b2 in range(BPC // 2):
                    emit_qkproj_group(oc, b2)
                    pump(2)
                    gctr += 1
                    if gctr > 16:
                        emit_v((gctr - 16) - v_pos[0])
                if oc < DC and b2 == BPC // 2 - 1:
                    for hx in (2 * hh, 2 * hh + 1):
                        for blk in range(4):
                            for kind in range(2):
                                pending_rel.append(make_rel_group(hx, blk, kind))

        # drain remaining rel groups and v
        pump(len(pending_rel))
        emit_v(len(v_groups))
        pa_cm.__exit__(None, None, None)
        pb_cm = tc.tile_pool(name="pb", bufs=3, space="PSUM")
        pb_pool = pb_cm.__enter__()

        # xT no longer needed; free its zone for the persistent o2 tile
        xt_pool_cm.__exit__(None, None, None)
        o2_pool = ctx.enter_context(tc.tile_pool(name="o2", bufs=1))
        o2_all = o2_pool.tile([128, DC, T], bf16)

        # ---- phase 3: attention, per head pair ----
        for hh in range(NH // 2):
            inflight = []

            def drain_one():
                pb, xa = inflight.pop(0)
                p_o = po_pool.tile([128, 2, 256], f32, tag="po")
                emit_av(pb, hh, xa, p_o)
                # hw: no divide ALU op, and TensorTensor reads at most one
                # input from PSUM -> reciprocal to SBUF, then multiply
                den_sb = attn_pool.tile([128, N], f32, tag="den")
                nc.vector.reciprocal(out=den_sb[:], in_=p_o[:, 1, 0:N])
                # (recip stays on DVE: Act is exp-saturated in attention)
                with nc.allow_low_precision(reason="bf16 softmax"):
                    nc.vector.tensor_tensor(
                        out=o2_all[:, hh, pb * N : (pb + 1) * N],
                        in0=p_o[:, 0, 0:N],
                        in1=den_sb[:],
                        op=ALU.mult,
                    )

            for b in range(BPC):
                a_sb = attn_pool.tile([128, 4, N], bf16, tag="a0")
                emit_qk_exp(b, hh, a_sb)
                pump(2)
                inflight.append((b, a_sb))
                if len(inflight) > 2:
                    drain_one()
            while inflight:
                drain_one()

        # ---- phase 4: projection over global 128-token chunks ----
        pb_cm.__exit__(None, None, None)
        ps4_cm = tc.tile_pool(name="ps4", bufs=2, space="PSUM")
        ps4 = ps4_cm.__enter__()
        NT_CH = (T + 127) // 128  # 13
        for j in range(NT_CH):
            t0 = j * 128
            tc_rows = min(128, T - t0)
            o_sb = osb_pool.tile([128, DIM], f32, tag="osb")
            for half in range(2):
                p_p = ps4.tile([128, 512], f32, tag="ps")
                for cc in range(DC):
                    nc.tensor.matmul(
                        p_p[0:tc_rows, 0:384],
                        lhsT=o2_all[:, cc, t0 : t0 + tc_rows],
                        rhs=pw_sb[:, cc, half * 384 : (half + 1) * 384],
                        start=(cc == 0),
                        stop=(cc == DC - 1),
                    )
                nc.vector.tensor_tensor(
                    out=o_sb[0:tc_rows, half * 384 : (half + 1) * 384],
                    in0=p_p[0:tc_rows, 0:384],
                    in1=bp_sb[0:tc_rows, half * 384 : (half + 1) * 384],
                    op=ALU.add,
                )
                nc.sync.dma_start(
                    out=out[t0 : t0 + tc_rows, half * 384 : (half + 1) * 384],
                    in_=o_sb[0:tc_rows, half * 384 : (half + 1) * 384],
                )
        ps4_cm.__exit__(None, None, None)

    nc.finalize()
    return nc


def _host_prep(inputs):
    bf16 = ml_dtypes.bfloat16
    f8 = ml_dtypes.float8_e4m3fn
    x = np.asarray(inputs["x"], np.float32)
    qkv_w = np.asarray(inputs["qkv_w"], np.float32)
    qkv_b = np.asarray(inputs["qkv_b"], np.float32)
    proj_w = np.asarray(inputs["proj_w"], np.float32)
    proj_b = np.asarray(inputs["proj_b"], np.float32)
    la_q = np.asarray(inputs["la_q"], np.float32)
    lb_q = np.asarray(inputs["lb_q"], np.float32)
    la_v = np.asarray(inputs["la_v"], np.float32)
    lb_v = np.asarray(inputs["lb_v"], np.float32)
    rel_pos_h = np.asarray(inputs["rel_pos_h"], np.float32)
    rel_pos_w = np.asarray(inputs["rel_pos_w"], np.float32)

    Wq = qkv_w[:DIM] + lb_q @ la_q
    Wk = qkv_w[DIM : 2 * DIM]
    Wv = qkv_w[2 * DIM :] + lb_v @ la_v

    s8 = 64.0
    if FP8_QK == 2:
        wqk8_host = np.ascontiguousarray(
            np.concatenate([s8 * SCALE * Wq, s8 * Wk], 0).T.astype(f8)
        )
        bqk_host = (
            s8 * np.concatenate([SCALE * qkv_b[:DIM], qkv_b[DIM : 2 * DIM]])
        ).astype(np.float32)
    elif FP8_QK == 1:
        wqk8_host = np.ascontiguousarray((s8 * Wk).T.astype(f8))
        wqk_host = np.ascontiguousarray((SCALE * Wq).T.astype(bf16))
        bqk_host = np.concatenate(
            [SCALE * qkv_b[:DIM], s8 * qkv_b[DIM : 2 * DIM]]
        ).astype(np.float32)
    else:
        wqk_host = np.ascontiguousarray(
            np.concatenate([SCALE * Wq, Wk], 0).T.astype(bf16)
        )
        bqk_host = np.concatenate(
            [SCALE * qkv_b[:DIM], qkv_b[DIM : 2 * DIM]]
        ).astype(np.float32)
    wv_host = np.ascontiguousarray(Wv.T.astype(bf16))
    pw_host = np.ascontiguousarray(proj_w.T.astype(bf16))
    bv_host = np.ascontiguousarray(qkv_b[2 * DIM :].astype(bf16))
    bp_host = np.ascontiguousarray(proj_b.astype(bf16))

    idx = np.arange(H)[:, None] - np.arange(H)[None, :] + (H - 1)
    Rh = rel_pos_h[idx]  # [qh, kh_j, hd]
    Rw = rel_pos_w[idx]  # [qw, kw_j, hd]
    rscale = s8 if FP8_QK else 1.0
    relh_host = np.ascontiguousarray(
        (rscale * Rh / SCALE).transpose(2, 0, 1).reshape(HD, N).astype(bf16)
    )
    relw_host = np.ascontiguousarray(
        (rscale * Rw / SCALE).transpose(2, 0, 1).reshape(HD, N).astype(bf16)
    )

    kt = np.arange(N)
    oh_kh = (kt[None, :] // W == np.arange(H)[:, None]).astype(bf16)  # [14, 196]
    oh_kw = (kt[None, :] % W == np.arange(W)[:, None]).astype(bf16)
    z18 = np.zeros((18, N), bf16)
    oh_e_host = np.ascontiguousarray(np.concatenate([oh_kh, z18, oh_kw], 0))
    oh_o_host = np.ascontiguousarray(
        np.concatenate([oh_kw, z18, oh_kh, z18], 0)
    )

    shared = {
        "wv": wv_host,
        "pw": pw_host,
        "bqk": bqk_host,
        "bv": bv_host,
        "bp": bp_host,
        "relh": relh_host,
        "relw": relw_host,
        "oh_e": oh_e_host,
        "oh_o": oh_o_host,
    }
    if FP8_QK:
        shared["wqk8"] = wqk8_host
    if FP8_QK < 2:
        shared["wqk"] = wqk_host

    x_flat = x.reshape(B_TOTAL, N, DIM)
    in_maps = []
    for c in range(NCORES):
        xc = x_flat[c * BPC : (c + 1) * BPC].reshape(BPC * N, DIM)
        xT_c = np.ascontiguousarray(xc.T.astype(bf16))
        m = dict(shared)
        m["xT"] = xT_c
        if FP8_QK:
            m["xT8"] = np.ascontiguousarray(xT_c.astype(f8))
        in_maps.append(m)
    return in_maps


def kernel(**inputs):
    from concourse import bass_utils

    if "nc" not in _NC_CACHE:
        _NC_CACHE["nc"] = build_module()
    nc = _NC_CACHE["nc"]
    in_maps = _host_prep(inputs)
    res = bass_utils.run_bass_kernel_spmd(
        nc, in_maps, core_ids=list(range(NCORES))
    )
    outs = [r["out"].reshape(BPC, H, W, DIM) for r in res.results]
    return np.concatenate(outs, 0)
